# revision 1
# baseline (speedup 1.0000x reference)
"""Trainium2 Bass kernel for nn_BertHungarianLoss.

Reference computation (M=8, V=128000, P=8!=40320):
    prob  = softmax(logits)                              [M, V]
    score[p] = sum_j prob[j, target[perms[p, j]]]        [P]
    best  = argmax(score)  (first max)
    tb    = target[perms[best]]                          [M]
    loss  = -log_softmax(logits)[j, tb[j]]               [M]
    returns (loss, tb)

Distribution over 8 NeuronCores:
  - softmax denominators are REPLICATED: every core reduces exp() over the
    full logits (collectives in this environment cost ~6-9us warm and pay a
    fixed ~45-70us subsystem startup per execution, so one fewer collective
    beats vocab-sharding the 4MB read).
  - permutation-parallel scoring: core k scores perms [5040k, 5040(k+1))
    via a one-hot/PE-matmul formulation (16 perms K-packed per column);
    per-core winners (score, global index for first-max tiebreak, and the
    winner's loss/target vectors) are combined with ONE AllGather; each
    core then selects the winning candidate locally.
  - a dependency-free warm-up AllGather fires first so the fixed collective
    startup overlaps the ~50us of local compute.

All compute (softmax stats, gather of target logits, permutation scoring,
argmax, CE loss) happens on device; the host only slices/stages inputs and
reads core 0's output.
"""

import numpy as np

import concourse.bacc as bacc
import concourse.bass as bass
import concourse.mybir as mybir
import concourse.tile as tile
from concourse.bass import IndirectOffsetOnAxis
from concourse.bass_utils import run_bass_kernel_spmd

M = 8
V = 128000
P = 40320            # 8!
NCORES = 8
VSL = V // NCORES    # 16000 vocab slice
PSL = P // NCORES    # 5040 perms per core
HALF = PSL // 2      # 2520 (two perms K-packed per matmul column)
NMM = 5              # matmuls of 504 columns each
NCOL = HALF // NMM   # 504

f32 = mybir.dt.float32
i32 = mybir.dt.int32
u8 = mybir.dt.uint8

AF = mybir.ActivationFunctionType
OP = mybir.AluOpType
AX = mybir.AxisListType

BIG = 1.0e9


def build_program(dbg=False, sim=False):
    nc = bacc.Bacc("TRN2", target_bir_lowering=False, debug=False,
                   num_devices=NCORES)

    # ---- I/O ----
    lgf = nc.dram_tensor("lgf", [M, V], f32, kind="ExternalInput").ap()
    tgt = nc.dram_tensor("tgt", [1, M], i32, kind="ExternalInput").ap()
    # indirect-DMA source view: flat [N, 1], coef=1; offsets are element
    # indices, one offset per output partition (HW-verified semantics).
    lgf_ind = lgf.rearrange("j v -> (j v)").unsqueeze(1)
    pv = nc.dram_tensor("pv", [128, HALF], u8, kind="ExternalInput").ap()
    pml = nc.dram_tensor("pml", [PSL, M], u8, kind="ExternalInput").ap()
    ivec = nc.dram_tensor("ivec", [128, 1], f32, kind="ExternalInput").ap()
    wsel = nc.dram_tensor("wsel", [128, 16], f32, kind="ExternalInput").ap()
    blk16 = nc.dram_tensor("blk16", [128, 16], f32, kind="ExternalInput").ap()
    pidx = nc.dram_tensor("pidx", [16, HALF // 8], f32, kind="ExternalInput").ap()
    jsel = nc.dram_tensor("jsel", [128, M], f32, kind="ExternalInput").ap()
    ex128 = nc.dram_tensor("ex128", [M, 128], f32, kind="ExternalInput").ap()
    ob = nc.dram_tensor("ob", [64, 1], i32, kind="ExternalInput").ap()
    iv64 = nc.dram_tensor("iv64", [1, 64], f32, kind="ExternalInput").ap()
    io8k = nc.dram_tensor("io8k", [1, M], f32, kind="ExternalInput").ap()
    eye = nc.dram_tensor("eye", [128, 128], f32, kind="ExternalInput").ap()
    o_loss = nc.dram_tensor("loss", [1, M], f32, kind="ExternalOutput").ap()
    o_tb = nc.dram_tensor("tbest", [1, M], i32, kind="ExternalOutput").ap()
    o_warm = nc.dram_tensor("warm", [1, M], f32, kind="ExternalOutput").ap()

    rg = [list(range(NCORES))]

    with tile.TileContext(nc) as tc:
        with tc.tile_pool(name="sb", bufs=1) as sb, \
             tc.tile_pool(name="dr", bufs=1, space="DRAM") as dr, \
             tc.tile_pool(name="ps", bufs=1, space="PSUM") as ps, \
             tc.tile_pool(name="psm", bufs=3, space="PSUM") as psm:

            # ---------- warm-up collective (absorbs ncfw cold start) ------
            # dummy first DMA warms the SWDGE completion path so the
            # cc0_in write (gating the collective trigger) posts fast
            cc0_in = dr.tile([1, M], f32)
            cc0_out = dr.tile([NCORES, M], f32)
            if sim:
                # CoreSim flags reads of uninitialized DRAM; on hardware the
                # warm-up's payload is irrelevant, so skip the write and let
                # the trigger fire with no dependencies.
                nc.gpsimd.dma_start(cc0_in[:], io8k)
            nc.gpsimd.collective_compute(
                "AllGather", OP.bypass, replica_groups=rg,
                ins=[cc0_in.opt()], outs=[cc0_out.opt()])
            # dummy DMA warms the cold SWDGE completion path for the gathers
            warmq = dr.tile([1, M], f32)
            nc.gpsimd.dma_start(warmq[:], io8k)

            # ---------- stage in ----------
            # sync stream: tiny warm DMA, then the big loads, then (last)
            # the AG1-dependent load so nothing queues behind a collective.
            io8k_t = sb.tile([1, M], f32)
            nc.sync.dma_start(io8k_t[:], io8k)
            CH = V // 128 * 8 // 2                        # 4000 cols/chunk
            L = sb.tile([128, 2 * CH], f32)               # [128, 8000]
            lgr_v = lgf.rearrange("j (s c) -> (j s) c", s=16)
            nc.sync.dma_start(L[:, 0:CH], lgr_v[:, 0:CH])
            nc.sync.dma_start(L[:, CH:2 * CH], lgr_v[:, CH:2 * CH])
            pv_t = sb.tile([128, HALF], u8)
            nc.sync.dma_start(pv_t[:], pv)
            pidx_t = sb.tile([16, HALF // 8], f32)
            nc.sync.dma_start(pidx_t[:], pidx)
            jsel_t = sb.tile([128, M], f32)
            nc.sync.dma_start(jsel_t[:], jsel)
            ex128_t = sb.tile([M, 128], f32)
            nc.sync.dma_start(ex128_t[:], ex128)
            eye_t = sb.tile([128, 128], f32)
            nc.sync.dma_start(eye_t[:], eye)
            one1 = eye_t[0:1, 0:1]
            # scalar (ACT) stream loads: small tensors needed early
            tgt_t = sb.tile([1, M], i32)
            nc.scalar.dma_start(tgt_t[:], tgt)
            ob_p = sb.tile([64, 1], i32)
            nc.scalar.dma_start(ob_p[:], ob)
            ivec_t = sb.tile([128, 1], f32)
            nc.scalar.dma_start(ivec_t[:], ivec)
            wsel_t = sb.tile([128, 16], f32)
            nc.scalar.dma_start(wsel_t[:], wsel)
            blk16_t = sb.tile([128, 16], f32)
            nc.scalar.dma_start(blk16_t[:], blk16)
            iv64_t = sb.tile([1, 64], f32)
            nc.scalar.dma_start(iv64_t[:], iv64)

            # ---------- full softmax denominators (replicated) ----------
            E = sb.tile([128, 2 * CH], f32)               # exp scratch
            acc2 = sb.tile([128, 2], f32)
            nc.scalar.activation(E[:, 0:CH], L[:, 0:CH], AF.Exp,
                                 accum_out=acc2[:, 0:1])
            nc.scalar.activation(E[:, CH:2 * CH], L[:, CH:2 * CH], AF.Exp,
                                 accum_out=acc2[:, 1:2])
            sums = sb.tile([128, 1], f32)
            nc.vector.tensor_reduce(sums[:], acc2[:], axis=AX.X, op=OP.add)

            # ---------- gather logits at target columns ----------
            # r-layout: r = i*8 + j (j fastest); indirect DMA: one element
            # offset per output partition.
            t8f = sb.tile([1, M], f32)
            nc.vector.tensor_copy(t8f[:], tgt_t[:])
            t64row = sb.tile([1, 64], f32)
            nc.vector.tensor_copy(
                t64row[:].rearrange("p (i j) -> p i j", j=8),
                t8f[:].unsqueeze(2).to_broadcast((1, 8, 8)))
            tps = ps.tile([64, 1], f32, tag="t1")
            nc.tensor.matmul(tps[:], t64row[:], one1, start=True, stop=True)
            tpi = sb.tile([64, 1], i32)
            nc.vector.tensor_copy(tpi[:], tps[:])
            offs_p = sb.tile([64, 1], i32)
            nc.vector.tensor_tensor(offs_p[:], ob_p[:], tpi[:], OP.add)
            T_p = sb.tile([64, 1], f32)
            nc.gpsimd.indirect_dma_start(
                T_p[:], None, lgf_ind,
                IndirectOffsetOnAxis(ap=offs_p[:], axis=0))
            Trow_ps = ps.tile([1, 64], f32, tag="t1")
            nc.tensor.matmul(Trow_ps[:], T_p[:], eye_t[0:64, 0:64],
                             start=True, stop=True)
            Trow = sb.tile([1, 64], f32)
            nc.vector.tensor_copy(Trow[:], Trow_ps[:])
            expTrow = sb.tile([1, 64], f32)
            nc.scalar.activation(expTrow[:], Trow_ps[:], AF.Exp)
            e128 = sb.tile([1, 128], f32)
            nc.vector.tensor_copy(
                e128[:].rearrange("p (h r) -> p h r", h=2),
                expTrow[:].unsqueeze(1).to_broadcast((1, 2, 64)))
            expT2_ps = ps.tile([128, 1], f32, tag="t1")
            nc.tensor.matmul(expT2_ps[:], e128[:], one1, start=True, stop=True)
            expT2 = sb.tile([128, 1], f32)
            nc.vector.tensor_copy(expT2[:], expT2_ps[:])

            # mw[c, m] = (pv[c, m] == i(c)) * exp(T[j(c), i(c)])  (pre-AG1)
            mw = sb.tile([128, HALF], f32)
            nc.vector.tensor_scalar(mw[:], pv_t[:], ivec_t[:], expT2[:],
                                    OP.is_equal, OP.mult)

            # ---------- pre-AG1 scoring contraction ----------
            # Y2[j+8h, m] = exp(T[j, sigma_p(j)]) for p = h*2520 + m
            Y2sb = sb.tile([16, HALF], f32)
            for u in range(NMM):
                psY = psm.tile([16, NCOL], f32, tag="pm")
                nc.tensor.matmul(psY[:], wsel_t[:],
                                 mw[:, u * NCOL:(u + 1) * NCOL],
                                 start=True, stop=True)
                if u % 2 == 0:
                    nc.vector.tensor_copy(Y2sb[:, u * NCOL:(u + 1) * NCOL], psY[:])
                else:
                    nc.scalar.copy(Y2sb[:, u * NCOL:(u + 1) * NCOL], psY[:])
            # K-pack: Y16[(j+8h)*8+b, m] = Y2[j+8h, b*315+m]
            Y16 = sb.tile([128, HALF // 8], f32)
            nc.sync.dma_start(Y16[:], Y2sb[:])

            # ---------- S_j, 1/S, log(S) via PE reductions ----------
            S8_ps = ps.tile([M, 1], f32, tag="t2")
            nc.tensor.matmul(S8_ps[:], jsel_t[:], sums[:], start=True, stop=True)
            S8sb = sb.tile([M, 1], f32)
            nc.vector.tensor_copy(S8sb[:], S8_ps[:])
            recipS_p = sb.tile([M, 1], f32)
            nc.vector.reciprocal(recipS_p[:], S8sb[:])
            S8row_ps = ps.tile([1, M], f32, tag="t3")
            nc.tensor.matmul(S8row_ps[:], S8sb[:], eye_t[0:M, 0:M],
                             start=True, stop=True)
            lseN = sb.tile([1, M], f32)
            nc.scalar.activation(lseN[:], S8row_ps[:], AF.Ln)
            # rec2[x] = 1/S_{j(x)} for x = (h*8+j)*8+b
            rec2_ps = ps.tile([128, 1], f32, tag="t4")
            nc.tensor.matmul(rec2_ps[:], ex128_t[:], recipS_p[:],
                             start=True, stop=True)
            rec2 = sb.tile([128, 1], f32)
            nc.vector.tensor_copy(rec2[:], rec2_ps[:])

            # ---------- permutation scoring (post-AG1: one matmul) -----
            # R16[x, 8h+b] = (h(x)==h && b(x)==b) / S_{j(x)}
            R16 = sb.tile([128, 16], f32)
            nc.vector.tensor_scalar(R16[:], blk16_t[:], rec2[:], None, OP.mult)
            scores_ps = ps.tile([16, HALF // 8], f32, tag="t5")
            nc.tensor.matmul(scores_ps[:], R16[:], Y16[:], start=True, stop=True)
            s16 = sb.tile([16, HALF // 8], f32)
            nc.vector.tensor_copy(s16[:], scores_ps[:])

            # ---------- local argmax (first-max) on [16, 315] ----------
            pack = sb.tile([16, 2], f32)
            nc.vector.tensor_reduce(pack[:, 0:1], s16[:], axis=AX.X, op=OP.max)
            e1 = sb.tile([16, HALF // 8], f32)
            nc.vector.tensor_scalar(e1[:], s16[:], pack[:, 0:1], BIG,
                                    OP.is_lt, OP.mult)
            e2 = sb.tile([16, HALF // 8], f32)
            nc.vector.tensor_tensor(e2[:], e1[:], pidx_t[:], OP.add)
            nc.vector.tensor_reduce(pack[:, 1:2], e2[:], axis=AX.X, op=OP.min)

            psA = ps.tile([1, 16], f32, tag="t3")
            psB = ps.tile([1, 16], f32, tag="t4")
            nc.tensor.matmul(psA[:], pack[:, 0:1], eye_t[0:16, 0:16],
                             start=True, stop=True)
            nc.tensor.matmul(psB[:], pack[:, 1:2], eye_t[0:16, 0:16],
                             start=True, stop=True)
            m_loc = sb.tile([1, 1], f32)
            nc.vector.tensor_reduce(m_loc[:], psA[:], axis=AX.X, op=OP.max)
            g1 = sb.tile([1, 16], f32)
            nc.vector.tensor_scalar(g1[:], psA[:], m_loc[:], BIG,
                                    OP.is_lt, OP.mult)
            g2 = sb.tile([1, 16], f32)
            nc.vector.tensor_tensor(g2[:], g1[:], psB[:], OP.add)
            i_loc = sb.tile([1, 1], f32)
            nc.vector.tensor_reduce(i_loc[:], g2[:], axis=AX.X, op=OP.min)

            # ---------- local candidate: loss/tb for this core's winner ----
            i_loc8 = sb.tile([1, 1], f32)
            nc.vector.tensor_scalar(i_loc8[:], i_loc[:], 8.0, None, OP.mult)
            pb_offf = sb.tile([1, M], f32)
            nc.vector.tensor_scalar(pb_offf[:], io8k_t[:], i_loc8[:], None, OP.add)
            pb_offi = sb.tile([1, M], i32)
            nc.vector.tensor_copy(pb_offi[:], pb_offf[:])
            pbrow = sb.tile([1, M], u8)
            nc.gpsimd.indirect_dma_start(
                pbrow[:], None, pml.rearrange("a b -> (a b)").unsqueeze(1),
                IndirectOffsetOnAxis(ap=pb_offi[:], axis=0))
            pbf = sb.tile([1, M], f32)
            nc.vector.tensor_copy(pbf[:], pbrow[:])

            # r = i*8 + j: mask[r] = (i(r) == perm_best[j(r)])
            mask = sb.tile([1, 64], f32)
            nc.vector.tensor_tensor(
                mask[:].rearrange("p (i j) -> p i j", j=8),
                iv64_t[:].rearrange("p (i j) -> p i j", j=8),
                pbf[:].unsqueeze(1).to_broadcast((1, 8, 8)), OP.is_equal)
            tm = sb.tile([1, 64], f32)
            nc.vector.tensor_tensor(tm[:], mask[:], Trow[:], OP.mult)
            Tb = sb.tile([1, M], f32)
            nc.vector.tensor_reduce(Tb[:],
                                    tm[:].rearrange("p (i j) -> p j i", j=8),
                                    axis=AX.X, op=OP.add)
            lcand = sb.tile([1, M], f32)
            nc.vector.tensor_tensor(lcand[:], lseN[:], Tb[:], OP.subtract)

            tgf = sb.tile([1, M], f32)
            nc.vector.tensor_copy(tgf[:], tgt_t[:])
            tm2 = sb.tile([1, 64], f32)
            nc.vector.tensor_tensor(
                tm2[:].rearrange("p (i j) -> p i j", j=8),
                mask[:].rearrange("p (i j) -> p i j", j=8),
                tgf[:].unsqueeze(2).to_broadcast((1, 8, 8)), OP.mult)
            tbc = sb.tile([1, M], f32)
            nc.vector.tensor_reduce(tbc[:],
                                    tm2[:].rearrange("p (i j) -> p j i", j=8),
                                    axis=AX.X, op=OP.add)

            cand = sb.tile([1, 24], f32)
            nc.vector.memset(cand[:], 0.0)
            nc.vector.tensor_copy(cand[:, 0:1], m_loc[:])
            nc.vector.tensor_copy(cand[:, 1:2], i_loc[:])
            nc.vector.tensor_copy(cand[:, 2:10], lcand[:])
            nc.vector.tensor_copy(cand[:, 10:18], tbc[:])

            # ---------- AllGather #2: candidates ----------
            cc2_in = dr.tile([1, 24], f32)
            cc2_out = dr.tile([NCORES, 24], f32)
            nc.gpsimd.dma_start(cc2_in[:], cand[:])
            nc.gpsimd.collective_compute(
                "AllGather", OP.bypass, replica_groups=rg,
                ins=[cc2_in.opt()], outs=[cc2_out.opt()])
            back2 = sb.tile([1, NCORES * 24], f32)
            i_back2 = nc.gpsimd.dma_start(
                back2[:].rearrange("p (r c) -> p r c", r=NCORES), cc2_out[:])
            b2 = back2[:].rearrange("p (r c) -> p r c", r=NCORES)
            scr = b2[:, :, 0]               # [1, 8] stride 24
            idxr = b2[:, :, 1]
            loss_all = b2[:, :, 2:10].transpose([0, 2, 1])   # [1, 8j, 8r]
            tb_all = b2[:, :, 10:18].transpose([0, 2, 1])

            m_fin = sb.tile([1, 1], f32)
            nc.vector.tensor_reduce(m_fin[:], scr, axis=AX.X, op=OP.max)
            f1 = sb.tile([1, NCORES], f32)
            nc.vector.tensor_scalar(f1[:], scr, m_fin[:], BIG, OP.is_lt, OP.mult)
            f2 = sb.tile([1, NCORES], f32)
            nc.vector.tensor_tensor(f2[:], f1[:], idxr, OP.add)
            i_fin = sb.tile([1, 1], f32)
            nc.vector.tensor_reduce(i_fin[:], f2[:], axis=AX.X, op=OP.min)
            sel = sb.tile([1, NCORES], f32)
            nc.vector.tensor_scalar(sel[:], f2[:], i_fin[:], None, OP.is_equal)

            lsel = sb.tile([1, 64], f32)
            nc.vector.tensor_tensor(
                lsel[:].rearrange("p (j r) -> p j r", r=8), loss_all,
                sel[:].unsqueeze(1).to_broadcast((1, 8, 8)), OP.mult)
            lossF = sb.tile([1, M], f32)
            nc.vector.tensor_reduce(lossF[:],
                                    lsel[:].rearrange("p (j r) -> p j r", r=8),
                                    axis=AX.X, op=OP.add)
            tsel = sb.tile([1, 64], f32)
            nc.gpsimd.tensor_tensor(
                tsel[:].rearrange("p (j r) -> p j r", r=8), tb_all,
                sel[:].unsqueeze(1).to_broadcast((1, 8, 8)), OP.mult)
            tbFf = sb.tile([1, M], f32)
            nc.vector.tensor_reduce(tbFf[:],
                                    tsel[:].rearrange("p (j r) -> p j r", r=8),
                                    axis=AX.X, op=OP.add)
            tbFi = sb.tile([1, M], i32)
            nc.vector.tensor_copy(tbFi[:], tbFf[:])

            nc.sync.dma_start(o_loss, lossF[:])
            nc.sync.dma_start(o_tb, tbFi[:])
            # consume the warm-up collective so it cannot be dead-coded;
            # ordered after the last real gpsimd DMA so the scheduler cannot
            # hoist it in front of work (it waits on the cold collective).
            i_warm = nc.gpsimd.dma_start(o_warm, cc0_out[0:1, :])
            from concourse.tile import add_dep_helper as _adh
            _adh(i_warm.ins, i_back2.ins, sync=True,
                 reason="warm-read must not block real gpsimd work")

            if dbg:
                def dump(name, t, shape):
                    o = nc.dram_tensor(name, shape, t.dtype,
                                       kind="ExternalOutput").ap()
                    nc.sync.dma_start(o, t)
                dump("d_sums", sums[:], [128, 1])
                dump("d_Trow", Trow[:], [1, 64])
                dump("d_expT2", expT2[:], [128, 1])
                dump("d_rec2", rec2[:], [128, 1])
                dump("d_mw", mw[:], [128, HALF])
                dump("d_iloc", i_loc[:], [1, 1])
                dump("d_cand", cand[:], [1, 24])
                dump("d_back2", back2[:], [1, NCORES * 24])

    nc.compile()
    return nc


_NC_CACHE = None


def _get_program():
    global _NC_CACHE
    if _NC_CACHE is None:
        _NC_CACHE = build_program()
    return _NC_CACHE


def make_in_maps(logits, target, perms):
    logits = np.ascontiguousarray(np.asarray(logits, dtype=np.float32))
    target = np.asarray(target).astype(np.int32).reshape(1, M)
    perms = np.asarray(perms).astype(np.int64)

    # r = i*8 + j convention: j(c) = c % 8, i(c) = (c % 64) // 8
    ivec = ((np.arange(128) % 64) // 8).astype(np.float32).reshape(128, 1)
    cc = np.arange(128)
    wsel = np.zeros((128, 16), dtype=np.float32)
    wsel[cc, (cc % 8) + 8 * (cc // 64)] = 1.0
    blk16 = np.zeros((128, 16), dtype=np.float32)
    blk16[cc, 8 * (cc // 64) + (cc % 8)] = 1.0
    ob = ((np.arange(64) % 8) * V).astype(np.int32).reshape(64, 1)
    iv64 = (np.arange(64) // 8).astype(np.float32).reshape(1, 64)

    c = np.arange(128)
    jc = c % 8                  # j(c)
    in_maps = []
    for k in range(NCORES):
        psl = perms[k * PSL:(k + 1) * PSL]              # [5040, 8]
        # pv[c, m] = perms_local[(c//64)*2520 + m, j(c)]
        half = (c // 64)
        pvk = psl[(half[:, None] * HALF + np.arange(HALF)[None, :]), jc[:, None]]
        in_maps.append({
            "lgf": logits,
            "tgt": target,
            "pv": pvk.astype(np.uint8),
            "pml": psl.astype(np.uint8),
            "ivec": ivec,
            "wsel": wsel,
            "blk16": blk16,
            "pidx": (k * PSL + np.arange(PSL)).astype(np.float32).reshape(16, 315),
            "ob": ob,
            "iv64": iv64,
            "io8k": (np.arange(8) - 8.0 * k * PSL).astype(np.float32).reshape(1, 8),
            "eye": np.eye(128, dtype=np.float32),
            "jsel": (np.arange(128)[:, None] // 16 == np.arange(8)[None, :]
                     ).astype(np.float32),
            "ex128": (np.arange(8)[:, None] == (np.arange(128)[None, :] % 64) // 8
                      ).astype(np.float32),
        })
    return in_maps


def run(logits, target, perms, trace=False):
    nc = _get_program()
    in_maps = make_in_maps(logits, target, perms)
    res = run_bass_kernel_spmd(nc, in_maps, core_ids=list(range(NCORES)),
                               trace=trace)
    loss = res.results[0]["loss"].reshape(M).astype(np.float32)
    tb = res.results[0]["tbest"].reshape(M).astype(np.int32)
    return loss, tb, res


def kernel(logits, target, perms):
    loss, tb, _ = run(logits, target, perms, trace=False)
    return loss, tb



# revision 8
# speedup vs baseline: 1.2169x; 1.2169x over previous
"""Trainium2 Bass kernel for nn_BertHungarianLoss — single-core version.

Reference computation (M=8, V=128000, P=8!=40320):
    prob  = softmax(logits)                              [M, V]
    score[p] = sum_j prob[j, target[perms[p, j]]]        [P]
    best  = argmax(score)  (first max)
    tb    = target[perms[best]]                          [M]
    loss  = -log_softmax(logits)[j, tb[j]]               [M]
    returns (loss, tb)

Why single core: on this part the collective subsystem costs ~60us per
execution (a ~44us start barrier plus AllGather trigger latency), while
the entire real workload is one 4MB logits read (~14us) plus small
matmuls.  Any multi-core split must pay the collective tax, so one core
with zero collectives wins by a wide margin.

Why the permutation scoring is cheap: score(p) factors over a
prefix/suffix split.  With w[j,i] = prob[j, target[i]],
    score(p) = A[p[0:4]] + B[p[4:8]],
where A/B are indexed by the 1680 ordered 4-tuples of distinct values.
Both tables come out of ONE 1680-column PE matmul (one-hot gather), and
argmax over all 40320 perms reduces to
    max_n (A[n] + maxB[complement(subset(n))]),
an O(1680) vector op.  The 24x reduction vs scoring all perms directly
keeps the PE off the critical path on a single core.

Index convention throughout (from the baseline kernel, HW-verified):
r = i*8 + j encodes the (i,j) pair of w[j, i] at SBUF partition r;
j = r % 8, i = (r % 64) // 8.
"""

import numpy as np
from itertools import permutations, combinations

import concourse.bacc as bacc
import concourse.mybir as mybir
import concourse.tile as tile
from concourse.bass import IndirectOffsetOnAxis
from concourse.bass_utils import run_bass_kernel_spmd

M = 8
V = 128000
NT = 1680            # ordered distinct 4-tuples of 8 values
NS = 70              # 4-subsets of 8
NCHUNK = 8           # logits DMA/exp chunks
CH = V * M // 128 // NCHUNK   # 1000 cols per chunk on the [128, 8000] view

f32 = mybir.dt.float32
f32r = mybir.dt.float32r
i32 = mybir.dt.int32
u8 = mybir.dt.uint8

AF = mybir.ActivationFunctionType
OP = mybir.AluOpType
AX = mybir.AxisListType

BIG = 1.0e9
MM_DT = f32          # dtype of the scoring matmul operands (f32 or f32r)


def build_program(dbg=False):
    nc = bacc.Bacc("TRN2", target_bir_lowering=False, debug=False,
                   num_devices=1)

    # ---- I/O ----
    lgf = nc.dram_tensor("lgf", [M, V], f32, kind="ExternalInput").ap()
    tgt = nc.dram_tensor("tgt", [1, M], i32, kind="ExternalInput").ap()
    # indirect-DMA source view: flat [N, 1]; offsets are element indices
    lgf_ind = lgf.rearrange("j v -> (j v)").unsqueeze(1)
    ohu = nc.dram_tensor("ohu", [128, NT], u8, kind="ExternalInput").ap()
    ohc = nc.dram_tensor("ohc", [NS, NS], MM_DT, kind="ExternalInput").ap()
    hmask = nc.dram_tensor("hmask", [128, 33], f32, kind="ExternalInput").ap()
    jsel = nc.dram_tensor("jsel", [128, M], f32, kind="ExternalInput").ap()
    eye = nc.dram_tensor("eye", [128, 128], f32, kind="ExternalInput").ap()
    ob = nc.dram_tensor("ob", [64, 1], i32, kind="ExternalInput").ap()
    iv64 = nc.dram_tensor("iv64", [1, 64], f32, kind="ExternalInput").ap()
    iotan = nc.dram_tensor("iotan", [1, NT], f32, kind="ExternalInput").ap()
    iotasn = nc.dram_tensor("iotasn", [1, NT], f32, kind="ExternalInput").ap()
    oidx = nc.dram_tensor("oidx", [1, NT], f32, kind="ExternalInput").ap()
    iotas = nc.dram_tensor("iotas", [1, NS], f32, kind="ExternalInput").ap()
    compf = nc.dram_tensor("compf", [1, NS], f32, kind="ExternalInput").ap()
    base4 = nc.dram_tensor("base4", [1, M], f32, kind="ExternalInput").ap()
    pfv = nc.dram_tensor("pfv", [NT, 4], u8, kind="ExternalInput").ap()
    o_loss = nc.dram_tensor("loss", [1, M], f32, kind="ExternalOutput").ap()
    o_tb = nc.dram_tensor("tbest", [1, M], i32, kind="ExternalOutput").ap()

    with tile.TileContext(nc) as tc:
        with tc.tile_pool(name="sb", bufs=1) as sb, \
             tc.tile_pool(name="ps", bufs=1, space="PSUM") as ps, \
             tc.tile_pool(name="psm", bufs=4, space="PSUM") as psm:

            # ---------- stage in ----------
            # sync queue: the big logits chunks FIRST so nothing delays them
            L = sb.tile([128, NCHUNK * CH], f32)          # [128, 8000]
            lgr_v = lgf.rearrange("j (s c) -> (j s) c", s=16)
            for k in range(NCHUNK):
                nc.sync.dma_start(L[:, k * CH:(k + 1) * CH],
                                  lgr_v[:, k * CH:(k + 1) * CH])
            # tensor queue: one-hot table (215KB) in parallel
            ohu_t = sb.tile([128, NT], u8)
            nc.gpsimd.dma_start(ohu_t[:], ohu)
            eye_t = sb.tile([128, 128], f32)
            nc.gpsimd.dma_start(eye_t[:], eye)
            one1 = eye_t[0:1, 0:1]
            # scalar queue: small tensors needed early
            tgt_t = sb.tile([1, M], i32)
            nc.scalar.dma_start(tgt_t[:], tgt)
            ob_p = sb.tile([64, 1], i32)
            nc.scalar.dma_start(ob_p[:], ob)
            jsel_t = sb.tile([128, M], f32)
            nc.scalar.dma_start(jsel_t[:], jsel)
            hmask_t = sb.tile([128, 33], f32)
            nc.scalar.dma_start(hmask_t[:], hmask)
            ohc_t = sb.tile([NS, NS], MM_DT)
            nc.scalar.dma_start(ohc_t[:], ohc)
            # vector queue: index tables for the argmax stage
            iotan_t = sb.tile([1, NT], f32)
            nc.scalar.dma_start(iotan_t[:], iotan)
            iotasn_t = sb.tile([1, NT], f32)
            nc.scalar.dma_start(iotasn_t[:], iotasn)
            oidx_t = sb.tile([1, NT], f32)
            nc.scalar.dma_start(oidx_t[:], oidx)
            iotas_t = sb.tile([1, NS], f32)
            nc.scalar.dma_start(iotas_t[:], iotas)
            compf_t = sb.tile([1, NS], f32)
            nc.scalar.dma_start(compf_t[:], compf)
            base4_t = sb.tile([1, M], f32)
            nc.scalar.dma_start(base4_t[:], base4)
            iv64_t = sb.tile([1, 64], f32)
            nc.scalar.dma_start(iv64_t[:], iv64)

            # dummy exp: pulls the ACT exp-table load into the DMA window
            scr8 = sb.tile([1, M], f32)
            nc.scalar.activation(scr8[:], jsel_t[0:1, :], AF.Exp)

            # ---------- softmax denominators (full vocab, chunked) ----------
            E = sb.tile([128, NCHUNK * CH], f32)
            acc = sb.tile([128, NCHUNK], f32)
            for k in range(NCHUNK):
                nc.scalar.activation(E[:, k * CH:(k + 1) * CH],
                                     L[:, k * CH:(k + 1) * CH], AF.Exp,
                                     accum_out=acc[:, k:k + 1])
            sums = sb.tile([128, 1], f32)
            nc.vector.tensor_reduce(sums[:], acc[:], axis=AX.X, op=OP.add)

            # ---------- gather logits at target columns ----------
            # T'[j, i] = logits[j, target[i]] at partition r = i*8 + j
            t8f = sb.tile([1, M], f32)
            nc.vector.tensor_copy(t8f[:], tgt_t[:])
            t64row = sb.tile([1, 64], f32)
            nc.vector.tensor_copy(
                t64row[:].rearrange("p (i j) -> p i j", j=8),
                t8f[:].unsqueeze(2).to_broadcast((1, 8, 8)))
            tps = ps.tile([64, 1], f32, tag="t1")
            nc.tensor.matmul(tps[:], t64row[:], one1, start=True, stop=True)
            tpi = sb.tile([64, 1], i32)
            nc.vector.tensor_copy(tpi[:], tps[:])
            offs_p = sb.tile([64, 1], i32)
            nc.vector.tensor_tensor(offs_p[:], ob_p[:], tpi[:], OP.add)
            T_p = sb.tile([64, 1], f32)
            nc.gpsimd.indirect_dma_start(
                T_p[:], None, lgf_ind,
                IndirectOffsetOnAxis(ap=offs_p[:], axis=0))
            Trow_ps = ps.tile([1, 64], f32, tag="t1")
            nc.tensor.matmul(Trow_ps[:], T_p[:], eye_t[0:64, 0:64],
                             start=True, stop=True)
            Trow = sb.tile([1, 64], f32)
            nc.vector.tensor_copy(Trow[:], Trow_ps[:])
            expTrow = sb.tile([1, 64], f32)
            nc.scalar.activation(expTrow[:], Trow_ps[:], AF.Exp)

            # one-hot table u8 -> matmul dtype (during the DMA window)
            ohf = sb.tile([128, NT], MM_DT)
            nc.vector.tensor_copy(ohf[:], ohu_t[:])

            # ---------- S_j, 1/S_j, log S_j ----------
            S8_ps = ps.tile([M, 1], f32, tag="t2")
            nc.tensor.matmul(S8_ps[:], jsel_t[:], sums[:], start=True, stop=True)
            S8sb = sb.tile([M, 1], f32)
            nc.vector.tensor_copy(S8sb[:], S8_ps[:])
            recipS_p = sb.tile([M, 1], f32)
            nc.vector.reciprocal(recipS_p[:], S8sb[:])
            S8row_ps = ps.tile([1, M], f32, tag="t3")
            nc.tensor.matmul(S8row_ps[:], S8sb[:], eye_t[0:M, 0:M],
                             start=True, stop=True)
            lseN = sb.tile([1, M], f32)
            nc.scalar.activation(lseN[:], S8row_ps[:], AF.Ln)
            rSrow_ps = ps.tile([1, M], f32, tag="t2")
            nc.tensor.matmul(rSrow_ps[:], recipS_p[:], eye_t[0:M, 0:M],
                             start=True, stop=True)
            rSrow = sb.tile([1, M], f32)
            nc.vector.tensor_copy(rSrow[:], rSrow_ps[:])

            # ---------- w = prob[j, target[i]] staged for the big matmul ----
            # w64[r] = exp(T'[j,i]) / S_j,  r = i*8 + j
            w64 = sb.tile([1, 64], f32)
            nc.vector.tensor_tensor(
                w64[:].rearrange("p (i j) -> p i j", j=8),
                expTrow[:].rearrange("p (i j) -> p i j", j=8),
                rSrow[:].unsqueeze(1).to_broadcast((1, 8, 8)), OP.mult)
            w128row = sb.tile([1, 128], f32)
            nc.vector.tensor_copy(
                w128row[:].rearrange("p (h r) -> p h r", h=2),
                w64[:].unsqueeze(1).to_broadcast((1, 2, 64)))
            w128_ps = ps.tile([128, 1], f32, tag="t1")
            nc.tensor.matmul(w128_ps[:], w128row[:], one1, start=True, stop=True)
            w128p = sb.tile([128, 1], f32)
            nc.vector.tensor_copy(w128p[:], w128_ps[:])
            # lhsT2[:, 0] = w masked to lower half (prefix: positions 0-3)
            # lhsT2[:, 1] = w masked to upper half (suffix: positions 4-7)
            lhsT2 = sb.tile([128, 33], MM_DT)
            nc.vector.tensor_scalar(lhsT2[:], hmask_t[:], w128p[:], None,
                                    OP.mult)

            # ---------- A/B tables: one 1680-column matmul ----------
            # AB[0, n] = A[n] (prefix score), AB[1, n] = B[n] (suffix score)
            Arow_t = sb.tile([1, NT], f32)
            Brow_t = sb.tile([1, NT], f32)
            NMM, MCOL = 4, NT // 4
            for u in range(NMM):
                psAB = psm.tile([33, MCOL], f32, tag="ab")
                nc.tensor.matmul(psAB[:], lhsT2[:],
                                 ohf[:, u * MCOL:(u + 1) * MCOL],
                                 start=True, stop=True)
                nc.vector.tensor_copy(Arow_t[:, u * MCOL:(u + 1) * MCOL],
                                      psAB[0:1, :])
                nc.vector.tensor_copy(Brow_t[:, u * MCOL:(u + 1) * MCOL],
                                      psAB[32:33, :])
            Arow = Arow_t[:]
            Brow = Brow_t[:]

            # ---------- per-subset suffix max + first-argmax ----------
            Bv = Brow.rearrange("p (s o) -> p s o", o=24)
            maxB = sb.tile([1, NS], f32)
            nc.vector.tensor_reduce(maxB[:], Bv, axis=AX.X, op=OP.max)
            blt = sb.tile([1, NT], f32)
            nc.vector.tensor_tensor(
                blt[:].rearrange("p (s o) -> p s o", o=24), Bv,
                maxB[:].unsqueeze(2).to_broadcast((1, NS, 24)), OP.is_lt)
            be = sb.tile([1, NT], f32)
            nc.vector.tensor_scalar(be[:], blt[:], BIG, None, OP.mult)
            nc.vector.tensor_tensor(be[:], be[:], oidx_t[:], OP.add)
            bo = sb.tile([1, NS], f32)
            nc.vector.tensor_reduce(bo[:],
                                    be[:].rearrange("p (s o) -> p s o", o=24),
                                    axis=AX.X, op=OP.min)

            # maxBc[s] = maxB[comp(s)] via tiny transpose + one-hot matmul
            mBp_ps = ps.tile([NS, 1], f32, tag="t1")
            nc.tensor.matmul(mBp_ps[:], maxB[:], one1, start=True, stop=True)
            mBp = sb.tile([NS, 1], MM_DT)
            nc.vector.tensor_copy(mBp[:], mBp_ps[:])
            mBc_ps = ps.tile([1, NS], f32, tag="t3")
            nc.tensor.matmul(mBc_ps[:], mBp[:], ohc_t[:], start=True, stop=True)
            maxBc = sb.tile([1, NS], f32)
            nc.vector.tensor_copy(maxBc[:], mBc_ps[:])

            # ---------- global argmax over tot[n] = A[n] + maxBc[s(n)] ------
            tot = sb.tile([1, NT], f32)
            nc.vector.tensor_tensor(
                tot[:].rearrange("p (s o) -> p s o", o=24),
                Arow.rearrange("p (s o) -> p s o", o=24),
                maxBc[:].unsqueeze(2).to_broadcast((1, NS, 24)), OP.add)
            mfin = sb.tile([1, 1], f32)
            nc.vector.tensor_reduce(mfin[:], tot[:], axis=AX.X, op=OP.max)
            e2 = sb.tile([1, NT], f32)
            nc.vector.tensor_scalar(e2[:], tot[:], mfin[:], BIG,
                                    OP.is_lt, OP.mult)
            e3 = sb.tile([1, NT], f32)
            nc.vector.tensor_tensor(e3[:], e2[:], iotasn_t[:], OP.add)
            nc.vector.tensor_tensor(e2[:], e2[:], iotan_t[:], OP.add)
            nstar = sb.tile([1, 1], f32)
            nc.vector.tensor_reduce(nstar[:], e2[:], axis=AX.X, op=OP.min)
            sstar = sb.tile([1, 1], f32)
            nc.vector.tensor_reduce(sstar[:], e3[:], axis=AX.X, op=OP.min)

            # cstar = comp(sstar); bostar = bo[cstar]; nB = cstar*24 + bostar
            eq1 = sb.tile([1, NS], f32)
            nc.vector.tensor_scalar(eq1[:], iotas_t[:], sstar[:], None,
                                    OP.is_equal)
            cm = sb.tile([1, NS], f32)
            nc.vector.tensor_tensor(cm[:], eq1[:], compf_t[:], OP.mult)
            cstar = sb.tile([1, 1], f32)
            nc.vector.tensor_reduce(cstar[:], cm[:], axis=AX.X, op=OP.add)
            eq2 = sb.tile([1, NS], f32)
            nc.vector.tensor_scalar(eq2[:], iotas_t[:], cstar[:], None,
                                    OP.is_equal)
            bm = sb.tile([1, NS], f32)
            nc.vector.tensor_tensor(bm[:], eq2[:], bo[:], OP.mult)
            bostar = sb.tile([1, 1], f32)
            nc.vector.tensor_reduce(bostar[:], bm[:], axis=AX.X, op=OP.add)

            # ---------- gather the winning perm's 8 values ----------
            na4 = sb.tile([1, 1], f32)
            nc.vector.tensor_scalar(na4[:], nstar[:], 4.0, None, OP.mult)
            nb4 = sb.tile([1, 1], f32)   # nb4 = (cstar*24 + bostar) * 4
            nc.vector.tensor_scalar(nb4[:], cstar[:], 24.0, None, OP.mult)
            nc.vector.tensor_tensor(nb4[:], nb4[:], bostar[:], OP.add)
            nc.vector.tensor_scalar(nb4[:], nb4[:], 4.0, None, OP.mult)
            offf = sb.tile([1, M], f32)
            nc.vector.tensor_scalar(offf[:, 0:4], base4_t[:, 0:4], na4[:],
                                    None, OP.add)
            nc.vector.tensor_scalar(offf[:, 4:8], base4_t[:, 4:8], nb4[:],
                                    None, OP.add)
            # one offset per destination partition: gather as [8, 1]
            offp_ps = ps.tile([M, 1], f32, tag="t2")
            nc.tensor.matmul(offp_ps[:], offf[:], one1, start=True, stop=True)
            offp = sb.tile([M, 1], i32)
            nc.vector.tensor_copy(offp[:], offp_ps[:])
            pb8 = sb.tile([M, 1], u8)
            nc.gpsimd.indirect_dma_start(
                pb8[:], None, pfv.rearrange("a b -> (a b)").unsqueeze(1),
                IndirectOffsetOnAxis(ap=offp[:], axis=0))
            pb8f = sb.tile([M, 1], f32)
            nc.vector.tensor_copy(pb8f[:], pb8[:])
            pbf_ps = ps.tile([1, M], f32, tag="t3")
            nc.tensor.matmul(pbf_ps[:], pb8f[:], eye_t[0:M, 0:M],
                             start=True, stop=True)
            pbf = sb.tile([1, M], f32)
            nc.vector.tensor_copy(pbf[:], pbf_ps[:])

            # ---------- loss and tb for the winning assignment ----------
            # mask[r] = (i(r) == perm_best[j(r)]),  r = i*8 + j
            mask = sb.tile([1, 64], f32)
            nc.vector.tensor_tensor(
                mask[:].rearrange("p (i j) -> p i j", j=8),
                iv64_t[:].rearrange("p (i j) -> p i j", j=8),
                pbf[:].unsqueeze(1).to_broadcast((1, 8, 8)), OP.is_equal)
            tm = sb.tile([1, 64], f32)
            nc.vector.tensor_tensor(tm[:], mask[:], Trow[:], OP.mult)
            Tb = sb.tile([1, M], f32)
            nc.vector.tensor_reduce(Tb[:],
                                    tm[:].rearrange("p (i j) -> p j i", j=8),
                                    axis=AX.X, op=OP.add)
            lossF = sb.tile([1, M], f32)
            nc.vector.tensor_tensor(lossF[:], lseN[:], Tb[:], OP.subtract)

            tgf = sb.tile([1, M], f32)
            nc.vector.tensor_copy(tgf[:], tgt_t[:])
            tm2 = sb.tile([1, 64], f32)
            nc.vector.tensor_tensor(
                tm2[:].rearrange("p (i j) -> p i j", j=8),
                mask[:].rearrange("p (i j) -> p i j", j=8),
                tgf[:].unsqueeze(2).to_broadcast((1, 8, 8)), OP.mult)
            tbc = sb.tile([1, M], f32)
            nc.vector.tensor_reduce(tbc[:],
                                    tm2[:].rearrange("p (i j) -> p j i", j=8),
                                    axis=AX.X, op=OP.add)
            tbFi = sb.tile([1, M], i32)
            nc.vector.tensor_copy(tbFi[:], tbc[:])

            nc.sync.dma_start(o_loss, lossF[:])
            nc.scalar.dma_start(o_tb, tbFi[:])

            if dbg:
                def dump(name, t, shape, dt=f32):
                    o = nc.dram_tensor(name, shape, t.dtype,
                                       kind="ExternalOutput").ap()
                    nc.sync.dma_start(o, t)
                dump("d_sums", sums[:], [128, 1])
                dump("d_Trow", Trow[:], [1, 64])
                dump("d_w64", w64[:], [1, 64])
                dump("d_A", Arow, [1, NT])
                dump("d_B", Brow, [1, NT])
                dump("d_maxB", maxB[:], [1, NS])
                dump("d_bo", bo[:], [1, NS])
                dump("d_maxBc", maxBc[:], [1, NS])
                dump("d_nstar", nstar[:], [1, 1])
                dump("d_sstar", sstar[:], [1, 1])
                dump("d_cstar", cstar[:], [1, 1])
                dump("d_nb4", nb4[:], [1, 1])
                dump("d_pbf", pbf[:], [1, M])
                dump("d_offf", offf[:], [1, M])
                dump("d_base4", base4_t[:], [1, M])
                dump("d_na4", na4[:], [1, 1])
                dump("d_bostar", bostar[:], [1, 1])

    nc.compile()
    return nc


_NC_CACHE = None


def _get_program():
    global _NC_CACHE
    if _NC_CACHE is None:
        _NC_CACHE = build_program()
    return _NC_CACHE


def _make_tables():
    subsets = list(combinations(range(8), 4))            # 70, lex order
    sidx = {s: i for i, s in enumerate(subsets)}
    comp = np.array([sidx[tuple(sorted(set(range(8)) - set(s)))]
                     for s in subsets], dtype=np.int64)
    tuples = []
    for s in subsets:
        for t in permutations(s):
            tuples.append(t)
    tuples = np.array(tuples, dtype=np.int64)            # [1680, 4]

    # one-hot gather table: col n = s*24+o
    #   lower rows  i*8 + j       (j = 0..3, i = tuples[n][j])
    #   upper rows  64 + i*8 + j  (j = 4..7, i = tuples[n][j-4])
    oh = np.zeros((128, NT), dtype=np.uint8)
    n = np.arange(NT)
    for jj in range(4):
        oh[tuples[:, jj] * 8 + jj, n] = 1
        oh[64 + tuples[:, jj] * 8 + (4 + jj), n] = 1

    ohc = np.zeros((NS, NS), dtype=np.float32)
    ohc[comp, np.arange(NS)] = 1.0                       # ohc[comp(s), s] = 1

    hmask = np.zeros((128, 33), dtype=np.float32)
    hmask[0:64, 0] = 1.0
    hmask[64:128, 32] = 1.0

    return {
        "ohu": oh,
        "ohc": ohc,
        "hmask": hmask,
        "iotan": np.arange(NT, dtype=np.float32).reshape(1, NT),
        "iotasn": (np.arange(NT) // 24).astype(np.float32).reshape(1, NT),
        "oidx": (np.arange(NT) % 24).astype(np.float32).reshape(1, NT),
        "iotas": np.arange(NS, dtype=np.float32).reshape(1, NS),
        "compf": comp.astype(np.float32).reshape(1, NS),
        "base4": np.array([0, 1, 2, 3, 0, 1, 2, 3],
                          dtype=np.float32).reshape(1, M),
        "pfv": tuples.astype(np.uint8),
        "jsel": (np.arange(128)[:, None] // 16 == np.arange(8)[None, :]
                 ).astype(np.float32),
        "eye": np.eye(128, dtype=np.float32),
        "ob": ((np.arange(64) % 8) * V).astype(np.int32).reshape(64, 1),
        "iv64": (np.arange(64) // 8).astype(np.float32).reshape(1, 64),
    }


_TABLES = None


def make_in_maps(logits, target, perms):
    global _TABLES
    if _TABLES is None:
        _TABLES = _make_tables()
    logits = np.ascontiguousarray(np.asarray(logits, dtype=np.float32))
    target = np.asarray(target).astype(np.int32).reshape(1, M)
    m = dict(_TABLES)
    m["lgf"] = logits
    m["tgt"] = target
    return [m]


def run(logits, target, perms, trace=False):
    nc = _get_program()
    in_maps = make_in_maps(logits, target, perms)
    res = run_bass_kernel_spmd(nc, in_maps, core_ids=[0], trace=trace)
    loss = res.results[0]["loss"].reshape(M).astype(np.float32)
    tb = res.results[0]["tbest"].reshape(M).astype(np.int32)
    return loss, tb, res


def kernel(logits, target, perms):
    loss, tb, _ = run(logits, target, perms, trace=False)
    return loss, tb


# revision 10
# speedup vs baseline: 2.2102x; 1.8162x over previous
"""Trainium2 Bass kernel for nn_BertHungarianLoss — single-core version.

Reference computation (M=8, V=128000, P=8!=40320):
    prob  = softmax(logits)                              [M, V]
    score[p] = sum_j prob[j, target[perms[p, j]]]        [P]
    best  = argmax(score)  (first max)
    tb    = target[perms[best]]                          [M]
    loss  = -log_softmax(logits)[j, tb[j]]               [M]
    returns (loss, tb)

Why single core: on this part the collective subsystem costs ~60us per
execution (a ~44us start barrier plus AllGather trigger latency), while
the entire real workload is one 4MB logits read (~14us) plus small
matmuls.  Any multi-core split must pay the collective tax, so one core
with zero collectives wins by a wide margin.

Scoring: score(p) factors over a prefix/suffix split.  With
w[j,i] = prob[j, target[i]],
    score(p) = A[p[0:4]] + B[p[4:8]],
and argmax over all 40320 perms reduces to
    max_n (A[n] + maxB[complement(subset(n))]).
A and B are materialized directly in [70 subsets (partitions), 24
orderings (free)] layout via a rank factorization:
    A[s, o] = sum_j w[j, elems(s)[rankperm_o(j)]]
            = sum_{(rho,j)} W4[(rho,j), s] * OH24A[(rho,j), o]
where W4[(rho,j), s] = w[j, elems(s)[rho]] is itself one one-hot matmul
from the 64 gathered w values.  Everything downstream (per-subset max,
argmax, complement lookup) then runs partition-parallel on the vector
engine instead of on a single partition.

Index convention (HW-verified in the baseline kernel): r = i*8 + j
encodes the (i,j) pair of w[j, i] at SBUF partition r; j = r % 8,
i = (r % 64) // 8, half = r // 64.
"""

import numpy as np
from itertools import permutations, combinations

import concourse.bacc as bacc
import concourse.mybir as mybir
import concourse.tile as tile
from concourse.bass import IndirectOffsetOnAxis
from concourse.bass_utils import run_bass_kernel_spmd

M = 8
V = 128000
NT = 1680            # ordered distinct 4-tuples of 8 values
NS = 70              # 4-subsets of 8
NCHUNK = 8           # logits DMA/exp chunks
CH = V * M // 128 // NCHUNK   # 1000 cols per chunk on the [128, 8000] view

f32 = mybir.dt.float32
i32 = mybir.dt.int32
u8 = mybir.dt.uint8

AF = mybir.ActivationFunctionType
OP = mybir.AluOpType
AX = mybir.AxisListType

BIG = 1.0e9

# f32 blob column layout (partition dim 128)
C_EYE = 0            # eye(128)                     [128, 128]
C_JSEL = 128         # jsel (S_j row groups)        [128, 8]
C_JH = 136           # jhmask (j,half selector)     [128, 16]
C_OHW1 = 152         # elem(s, rank=half)           [128, 70]
C_OHW2 = 222         # elem(s, rank=2+half)         [128, 70]
C_O24A = 292         # rank one-hot, positions 0-3  [64, 24]
C_O24B = 316         # rank one-hot, positions 4-7  [64, 24]
C_OHC = 340          # complement one-hot           [70, 70]
C_OIDX = 410         # oidx24[s,o] = o              [70, 24]
C_I70 = 434          # iota70 row                   [1, 70]
C_CMP = 504          # comp(s) row                  [1, 70]
C_IV64 = 574         # i-index row (r//8)           [1, 64]
C_B4 = 638           # base4 row                    [1, 8]
C_REP8 = 646         # rep8[k, q] = [k == q%8]      [8, 64]
NBLOB = 710


def build_program(dbg=False):
    nc = bacc.Bacc("TRN2", target_bir_lowering=False, debug=False,
                   num_devices=1)

    # ---- I/O ----
    lgf = nc.dram_tensor("lgf", [M, V], f32, kind="ExternalInput").ap()
    lgf_ind = lgf.rearrange("j v -> (j v)").unsqueeze(1)
    blob = nc.dram_tensor("blob", [128, NBLOB], f32, kind="ExternalInput").ap()
    iblob = nc.dram_tensor("iblob", [64, 9], i32, kind="ExternalInput").ap()
    pfv = nc.dram_tensor("pfv", [NT, 4], u8, kind="ExternalInput").ap()
    o_loss = nc.dram_tensor("loss", [1, M], f32, kind="ExternalOutput").ap()
    o_tb = nc.dram_tensor("tbest", [1, M], i32, kind="ExternalOutput").ap()

    with tile.TileContext(nc) as tc:
        with tc.tile_pool(name="sb", bufs=1) as sb, \
             tc.tile_pool(name="ps", bufs=1, space="PSUM") as ps:

            # ---------- stage in ----------
            # sync queue: the big logits chunks, nothing else before them
            L = sb.tile([128, NCHUNK * CH], f32)          # [128, 8000]
            lgr_v = lgf.rearrange("j (s c) -> (j s) c", s=16)
            for k in range(NCHUNK):
                nc.sync.dma_start(L[:, k * CH:(k + 1) * CH],
                                  lgr_v[:, k * CH:(k + 1) * CH])
            # gpsimd queue: the two table blobs
            B = sb.tile([128, NBLOB], f32)
            nc.gpsimd.dma_start(B[:], blob)
            ib = sb.tile([64, 9], i32)
            nc.gpsimd.dma_start(ib[:], iblob)

            one1 = B[0:1, C_EYE:C_EYE + 1]
            eye8 = B[0:M, C_EYE:C_EYE + M]
            eye64 = B[0:64, C_EYE:C_EYE + 64]
            eye70 = B[0:NS, C_EYE:C_EYE + NS]
            jsel_v = B[:, C_JSEL:C_JSEL + M]
            jh_v = B[:, C_JH:C_JH + 16]
            ohw1_v = B[:, C_OHW1:C_OHW1 + NS]
            ohw2_v = B[:, C_OHW2:C_OHW2 + NS]
            o24a_v = B[0:64, C_O24A:C_O24A + 24]
            o24b_v = B[0:64, C_O24B:C_O24B + 24]
            ohc_v = B[0:NS, C_OHC:C_OHC + NS]
            oidx_v = B[0:NS, C_OIDX:C_OIDX + 24]
            i70_v = B[0:1, C_I70:C_I70 + NS]
            cmp_v = B[0:1, C_CMP:C_CMP + NS]
            iv64_v = B[0:1, C_IV64:C_IV64 + 64]
            b4_v = B[0:1, C_B4:C_B4 + M]
            rep8_v = B[0:8, C_REP8:C_REP8 + 64]
            tgt_v = ib[0:1, 1:9]
            ob_v = ib[:, 0:1]

            # dummy exp on zeroed scratch: pull the ACT exp-table load
            # into the DMA window
            scr8 = sb.tile([1, M], f32)
            nc.vector.memset(scr8[:], 0.0)
            nc.scalar.activation(scr8[:], scr8[:], AF.Exp)

            # ---------- exp chunks 0-2 (scalar queue) ----------
            E = sb.tile([128, NCHUNK * CH], f32)
            acc = sb.tile([128, NCHUNK], f32)

            def exp_chunk(k):
                nc.scalar.activation(E[:, k * CH:(k + 1) * CH],
                                     L[:, k * CH:(k + 1) * CH], AF.Exp,
                                     accum_out=acc[:, k:k + 1])

            for k in range(3):
                exp_chunk(k)

            # ---------- gather logits at target columns ----------
            # T'[j, i] = logits[j, target[i]] at partition r = i*8 + j
            t8f = sb.tile([1, M], f32)
            nc.vector.tensor_copy(t8f[:], tgt_v)
            t64row = sb.tile([1, 64], f32)
            nc.vector.tensor_copy(
                t64row[:].rearrange("p (i j) -> p i j", j=8),
                t8f[:].unsqueeze(2).to_broadcast((1, 8, 8)))
            tps = ps.tile([64, 1], f32, tag="t1")
            nc.tensor.matmul(tps[:], t64row[:], one1, start=True, stop=True)
            tpi = sb.tile([64, 1], i32)
            nc.vector.tensor_copy(tpi[:], tps[:])
            offs_p = sb.tile([64, 1], i32)
            nc.vector.tensor_tensor(offs_p[:], ob_v, tpi[:], OP.add)
            T_p = sb.tile([64, 1], f32)
            nc.gpsimd.indirect_dma_start(
                T_p[:], None, lgf_ind,
                IndirectOffsetOnAxis(ap=offs_p[:], axis=0))
            Trow_ps = ps.tile([1, 64], f32, tag="t2")
            nc.tensor.matmul(Trow_ps[:], T_p[:], eye64, start=True, stop=True)
            Trow = sb.tile([1, 64], f32)
            nc.vector.tensor_copy(Trow[:], Trow_ps[:])
            # expTrow inserted on the scalar queue here so it lands while
            # the remaining chunks still stream
            expTrow = sb.tile([1, 64], f32)
            nc.scalar.activation(expTrow[:], Trow_ps[:], AF.Exp)

            for k in range(3, NCHUNK):
                exp_chunk(k)

            # ---------- unnormalized W4 table (hidden under the DMA) ----
            # W4[(rho,j), s] = exp(T'[j, elems(s)[rho]])
            e128row = sb.tile([1, 128], f32)
            nc.vector.tensor_copy(
                e128row[:].rearrange("p (h r) -> p h r", h=2),
                expTrow[:].unsqueeze(1).to_broadcast((1, 2, 64)))
            eT128_ps = ps.tile([128, 1], f32, tag="t1")
            nc.tensor.matmul(eT128_ps[:], e128row[:], one1,
                             start=True, stop=True)
            eT128 = sb.tile([128, 1], f32)
            nc.vector.tensor_copy(eT128[:], eT128_ps[:])
            WJe = sb.tile([128, 16], f32)
            nc.vector.tensor_scalar(WJe[:], jh_v, eT128[:], None, OP.mult)
            psW1 = ps.tile([16, NS], f32, tag="w1")
            nc.tensor.matmul(psW1[:], WJe[:], ohw1_v, start=True, stop=True)
            psW2 = ps.tile([16, NS], f32, tag="w2")
            nc.tensor.matmul(psW2[:], WJe[:], ohw2_v, start=True, stop=True)
            Wraw = sb.tile([64, NS], f32)
            # rows 16-31 / 48-63 are dead but must be finite: OH24's zero
            # rows would still propagate NaN through the PE accumulate
            nc.vector.memset(Wraw[:], 0.0)
            nc.vector.tensor_copy(Wraw[0:16, :], psW1[:])
            nc.vector.tensor_copy(Wraw[32:48, :], psW2[:])

            # ---------- S_j, 1/S_j, log S_j ----------
            sums = sb.tile([128, 1], f32)
            nc.vector.tensor_reduce(sums[:], acc[:], axis=AX.X, op=OP.add)
            S8_ps = ps.tile([M, 1], f32, tag="t3")
            nc.tensor.matmul(S8_ps[:], jsel_v, sums[:], start=True, stop=True)
            S8sb = sb.tile([M, 1], f32)
            nc.vector.tensor_copy(S8sb[:], S8_ps[:])
            recipS_p = sb.tile([M, 1], f32)
            nc.vector.reciprocal(recipS_p[:], S8sb[:])
            S8row_ps = ps.tile([1, M], f32, tag="t4")
            nc.tensor.matmul(S8row_ps[:], S8sb[:], eye8, start=True, stop=True)
            lseN = sb.tile([1, M], f32)
            nc.scalar.activation(lseN[:], S8row_ps[:], AF.Ln)

            # ---------- normalize W4 and emit A/B in [70, 24] ----------
            rec64_ps = ps.tile([64, 1], f32, tag="t3")
            nc.tensor.matmul(rec64_ps[:], rep8_v, recipS_p[:],
                             start=True, stop=True)
            rec64 = sb.tile([64, 1], f32)
            nc.vector.tensor_copy(rec64[:], rec64_ps[:])
            W4 = sb.tile([64, NS], f32)
            nc.vector.tensor_scalar(W4[:], Wraw[:], rec64[:], None, OP.mult)
            psA = ps.tile([NS, 24], f32, tag="a70")
            nc.tensor.matmul(psA[:], W4[:], o24a_v, start=True, stop=True)
            psB = ps.tile([NS, 24], f32, tag="b70")
            nc.tensor.matmul(psB[:], W4[:], o24b_v, start=True, stop=True)
            A70 = sb.tile([NS, 24], f32)
            nc.vector.tensor_copy(A70[:], psA[:])
            B70 = sb.tile([NS, 24], f32)
            nc.vector.tensor_copy(B70[:], psB[:])

            # ---------- per-subset suffix max + first-argmax ----------
            maxB = sb.tile([NS, 1], f32)
            nc.vector.tensor_reduce(maxB[:], B70[:], axis=AX.X, op=OP.max)
            boE = sb.tile([NS, 24], f32)
            nc.vector.tensor_scalar(boE[:], B70[:], maxB[:], BIG,
                                    OP.is_lt, OP.mult)
            nc.vector.tensor_tensor(boE[:], boE[:], oidx_v, OP.add)
            bo = sb.tile([NS, 1], f32)
            nc.vector.tensor_reduce(bo[:], boE[:], axis=AX.X, op=OP.min)

            # maxBc[s] = maxB[comp(s)]
            mBc_ps = ps.tile([NS, 1], f32, tag="t3")
            nc.tensor.matmul(mBc_ps[:], ohc_v, maxB[:], start=True, stop=True)
            maxBc = sb.tile([NS, 1], f32)
            nc.vector.tensor_copy(maxBc[:], mBc_ps[:])

            # ---------- tot = A + maxBc; row maxima and argmaxes ----------
            tot = sb.tile([NS, 24], f32)
            nc.vector.tensor_scalar(tot[:], A70[:], maxBc[:], None, OP.add)
            rmax = sb.tile([NS, 1], f32)
            nc.vector.tensor_reduce(rmax[:], tot[:], axis=AX.X, op=OP.max)
            oE = sb.tile([NS, 24], f32)
            nc.vector.tensor_scalar(oE[:], tot[:], rmax[:], BIG,
                                    OP.is_lt, OP.mult)
            nc.vector.tensor_tensor(oE[:], oE[:], oidx_v, OP.add)
            oarg = sb.tile([NS, 1], f32)
            nc.vector.tensor_reduce(oarg[:], oE[:], axis=AX.X, op=OP.min)

            # cross-partition: transpose the three [70,1] columns to rows
            rmT_ps = ps.tile([1, NS], f32, tag="t4")
            nc.tensor.matmul(rmT_ps[:], rmax[:], eye70, start=True, stop=True)
            rmT = sb.tile([1, NS], f32)
            nc.vector.tensor_copy(rmT[:], rmT_ps[:])
            oaT_ps = ps.tile([1, NS], f32, tag="t1")
            nc.tensor.matmul(oaT_ps[:], oarg[:], eye70, start=True, stop=True)
            oaT = sb.tile([1, NS], f32)
            nc.vector.tensor_copy(oaT[:], oaT_ps[:])
            boT_ps = ps.tile([1, NS], f32, tag="t2")
            nc.tensor.matmul(boT_ps[:], bo[:], eye70, start=True, stop=True)
            boT = sb.tile([1, NS], f32)
            nc.vector.tensor_copy(boT[:], boT_ps[:])

            # global first-max over subsets
            mfin = sb.tile([1, 1], f32)
            nc.vector.tensor_reduce(mfin[:], rmT[:], axis=AX.X, op=OP.max)
            es = sb.tile([1, NS], f32)
            nc.vector.tensor_scalar(es[:], rmT[:], mfin[:], BIG,
                                    OP.is_lt, OP.mult)
            nc.vector.tensor_tensor(es[:], es[:], i70_v, OP.add)
            sstar = sb.tile([1, 1], f32)
            nc.vector.tensor_reduce(sstar[:], es[:], axis=AX.X, op=OP.min)

            # ostar = oarg[sstar]; cstar = comp[sstar]; bostar = bo[cstar]
            eq1 = sb.tile([1, NS], f32)
            nc.vector.tensor_scalar(eq1[:], i70_v, sstar[:], None, OP.is_equal)
            g1 = sb.tile([1, NS], f32)
            nc.vector.tensor_tensor(g1[:], eq1[:], oaT[:], OP.mult)
            ostar = sb.tile([1, 1], f32)
            nc.vector.tensor_reduce(ostar[:], g1[:], axis=AX.X, op=OP.add)
            g2 = sb.tile([1, NS], f32)
            nc.vector.tensor_tensor(g2[:], eq1[:], cmp_v, OP.mult)
            cstar = sb.tile([1, 1], f32)
            nc.vector.tensor_reduce(cstar[:], g2[:], axis=AX.X, op=OP.add)
            eq2 = sb.tile([1, NS], f32)
            nc.vector.tensor_scalar(eq2[:], i70_v, cstar[:], None, OP.is_equal)
            g3 = sb.tile([1, NS], f32)
            nc.vector.tensor_tensor(g3[:], eq2[:], boT[:], OP.mult)
            bostar = sb.tile([1, 1], f32)
            nc.vector.tensor_reduce(bostar[:], g3[:], axis=AX.X, op=OP.add)

            # ---------- winning tuple indices -> byte offsets ----------
            # na4 = (sstar*24 + ostar)*4 ; nb4 = (cstar*24 + bostar)*4
            na4 = sb.tile([1, 1], f32)
            nc.vector.tensor_scalar(na4[:], sstar[:], 24.0, None, OP.mult)
            nc.vector.tensor_tensor(na4[:], na4[:], ostar[:], OP.add)
            nc.vector.tensor_scalar(na4[:], na4[:], 4.0, None, OP.mult)
            nb4 = sb.tile([1, 1], f32)
            nc.vector.tensor_scalar(nb4[:], cstar[:], 24.0, None, OP.mult)
            nc.vector.tensor_tensor(nb4[:], nb4[:], bostar[:], OP.add)
            nc.vector.tensor_scalar(nb4[:], nb4[:], 4.0, None, OP.mult)
            offf = sb.tile([1, M], f32)
            nc.vector.tensor_scalar(offf[:, 0:4], b4_v[:, 0:4], na4[:],
                                    None, OP.add)
            nc.vector.tensor_scalar(offf[:, 4:8], b4_v[:, 4:8], nb4[:],
                                    None, OP.add)
            offp_ps = ps.tile([M, 1], f32, tag="t3")
            nc.tensor.matmul(offp_ps[:], offf[:], one1, start=True, stop=True)
            offp = sb.tile([M, 1], i32)
            nc.vector.tensor_copy(offp[:], offp_ps[:])
            pb8 = sb.tile([M, 1], u8)
            nc.gpsimd.indirect_dma_start(
                pb8[:], None, pfv.rearrange("a b -> (a b)").unsqueeze(1),
                IndirectOffsetOnAxis(ap=offp[:], axis=0))
            pb8f = sb.tile([M, 1], f32)
            nc.vector.tensor_copy(pb8f[:], pb8[:])
            pbf_ps = ps.tile([1, M], f32, tag="t4")
            nc.tensor.matmul(pbf_ps[:], pb8f[:], eye8, start=True, stop=True)
            pbf = sb.tile([1, M], f32)
            nc.vector.tensor_copy(pbf[:], pbf_ps[:])

            # ---------- loss and tb for the winning assignment ----------
            # mask[r] = (i(r) == perm_best[j(r)]),  r = i*8 + j
            mask = sb.tile([1, 64], f32)
            nc.vector.tensor_tensor(
                mask[:].rearrange("p (i j) -> p i j", j=8),
                iv64_v.rearrange("p (i j) -> p i j", j=8),
                pbf[:].unsqueeze(1).to_broadcast((1, 8, 8)), OP.is_equal)
            tm = sb.tile([1, 64], f32)
            nc.vector.tensor_tensor(tm[:], mask[:], Trow[:], OP.mult)
            Tb = sb.tile([1, M], f32)
            nc.vector.tensor_reduce(Tb[:],
                                    tm[:].rearrange("p (i j) -> p j i", j=8),
                                    axis=AX.X, op=OP.add)
            lossF = sb.tile([1, M], f32)
            nc.vector.tensor_tensor(lossF[:], lseN[:], Tb[:], OP.subtract)

            tgf = sb.tile([1, M], f32)
            nc.vector.tensor_copy(tgf[:], tgt_v)
            tm2 = sb.tile([1, 64], f32)
            nc.vector.tensor_tensor(
                tm2[:].rearrange("p (i j) -> p i j", j=8),
                mask[:].rearrange("p (i j) -> p i j", j=8),
                tgf[:].unsqueeze(2).to_broadcast((1, 8, 8)), OP.mult)
            tbc = sb.tile([1, M], f32)
            nc.vector.tensor_reduce(tbc[:],
                                    tm2[:].rearrange("p (i j) -> p j i", j=8),
                                    axis=AX.X, op=OP.add)
            tbFi = sb.tile([1, M], i32)
            nc.vector.tensor_copy(tbFi[:], tbc[:])

            nc.sync.dma_start(o_loss, lossF[:])
            nc.scalar.dma_start(o_tb, tbFi[:])

            if dbg:
                def dump(name, t, shape):
                    o = nc.dram_tensor(name, shape, t.dtype,
                                       kind="ExternalOutput").ap()
                    nc.sync.dma_start(o, t)
                dump("d_sums", sums[:], [128, 1])
                dump("d_Trow", Trow[:], [1, 64])
                dump("d_W4", W4[:], [64, NS])
                dump("d_A70", A70[:], [NS, 24])
                dump("d_B70", B70[:], [NS, 24])
                dump("d_maxB", maxB[:], [NS, 1])
                dump("d_bo", bo[:], [NS, 1])
                dump("d_maxBc", maxBc[:], [NS, 1])
                dump("d_sstar", sstar[:], [1, 1])
                dump("d_ostar", ostar[:], [1, 1])
                dump("d_cstar", cstar[:], [1, 1])
                dump("d_bostar", bostar[:], [1, 1])
                dump("d_offf", offf[:], [1, M])
                dump("d_pbf", pbf[:], [1, M])

    nc.compile()
    return nc


_NC_CACHE = None


def _get_program():
    global _NC_CACHE
    if _NC_CACHE is None:
        _NC_CACHE = build_program()
    return _NC_CACHE


def _make_tables():
    subsets = list(combinations(range(8), 4))            # 70, lex order
    sidx = {s: i for i, s in enumerate(subsets)}
    comp = np.array([sidx[tuple(sorted(set(range(8)) - set(s)))]
                     for s in subsets], dtype=np.int64)
    elems = np.array(subsets, dtype=np.int64)            # [70, 4] sorted
    rp = np.array(list(permutations(range(4))), dtype=np.int64)  # [24, 4]
    tuples = []
    for s in subsets:
        for t in permutations(s):
            tuples.append(t)
    tuples = np.array(tuples, dtype=np.int64)            # [1680, 4]

    r = np.arange(128)
    jr = r % 8
    ir = (r % 64) // 8
    hr = r // 64

    blob = np.zeros((128, NBLOB), dtype=np.float32)
    blob[:, C_EYE:C_EYE + 128] = np.eye(128, dtype=np.float32)
    blob[:, C_JSEL:C_JSEL + M] = (
        np.arange(128)[:, None] // 16 == np.arange(8)[None, :])
    # jhmask[r, q] = [j(r) == q%8] * [half(r) == q//8]
    q = np.arange(16)
    blob[:, C_JH:C_JH + 16] = (
        (jr[:, None] == (q % 8)[None, :]) & (hr[:, None] == (q // 8)[None, :]))
    # OHW1[r, s] = [i(r) == elems(s)[half(r)]]
    blob[:, C_OHW1:C_OHW1 + NS] = (
        ir[:, None] == elems[:, 0:2].T[hr, :])
    # OHW2[r, s] = [i(r) == elems(s)[2 + half(r)]]
    blob[:, C_OHW2:C_OHW2 + NS] = (
        ir[:, None] == elems[:, 2:4].T[hr, :])
    # OH24A/B rows: W4 row layout r<16: rho=r//8, j=r%8;
    #               32<=r<48: rho=2+(r-32)//8, j=(r-32)%8; else dead.
    o24a = np.zeros((64, 24), dtype=np.float32)
    o24b = np.zeros((64, 24), dtype=np.float32)
    for rr in range(64):
        if rr < 16:
            rho, j = rr // 8, rr % 8
        elif 32 <= rr < 48:
            rho, j = 2 + (rr - 32) // 8, (rr - 32) % 8
        else:
            continue
        if j <= 3:
            o24a[rr, :] = (rp[:, j] == rho)
        else:
            o24b[rr, :] = (rp[:, j - 4] == rho)
    blob[0:64, C_O24A:C_O24A + 24] = o24a
    blob[0:64, C_O24B:C_O24B + 24] = o24b
    # ohc[k, s] = [k == comp(s)]
    ohc = np.zeros((NS, NS), dtype=np.float32)
    ohc[comp, np.arange(NS)] = 1.0
    blob[0:NS, C_OHC:C_OHC + NS] = ohc
    blob[0:NS, C_OIDX:C_OIDX + 24] = np.arange(24)[None, :]
    blob[0, C_I70:C_I70 + NS] = np.arange(NS)
    blob[0, C_CMP:C_CMP + NS] = comp
    blob[0, C_IV64:C_IV64 + 64] = np.arange(64) // 8
    blob[0, C_B4:C_B4 + M] = [0, 1, 2, 3, 0, 1, 2, 3]
    blob[0:8, C_REP8:C_REP8 + 64] = (
        np.arange(8)[:, None] == (np.arange(64) % 8)[None, :])

    return blob, tuples.astype(np.uint8)


_TABLES = None


def make_in_maps(logits, target, perms):
    global _TABLES
    if _TABLES is None:
        _TABLES = _make_tables()
    blob, pfv = _TABLES
    logits = np.ascontiguousarray(np.asarray(logits, dtype=np.float32))
    target = np.asarray(target).astype(np.int64).reshape(M)
    iblob = np.zeros((64, 9), dtype=np.int32)
    iblob[:, 0] = (np.arange(64) % 8) * V
    iblob[0, 1:9] = target
    return [{"lgf": logits, "blob": blob, "iblob": iblob, "pfv": pfv}]


def run(logits, target, perms, trace=False):
    nc = _get_program()
    in_maps = make_in_maps(logits, target, perms)
    res = run_bass_kernel_spmd(nc, in_maps, core_ids=[0], trace=trace)
    loss = res.results[0]["loss"].reshape(M).astype(np.float32)
    tb = res.results[0]["tbest"].reshape(M).astype(np.int32)
    return loss, tb, res


def kernel(logits, target, perms):
    loss, tb, _ = run(logits, target, perms, trace=False)
    return loss, tb


# revision 13
# speedup vs baseline: 2.2855x; 1.0340x over previous
"""Trainium2 Bass kernel for nn_BertHungarianLoss — single-core version.

Reference computation (M=8, V=128000, P=8!=40320):
    prob  = softmax(logits)                              [M, V]
    score[p] = sum_j prob[j, target[perms[p, j]]]        [P]
    best  = argmax(score)  (first max)
    tb    = target[perms[best]]                          [M]
    loss  = -log_softmax(logits)[j, tb[j]]               [M]
    returns (loss, tb)

Why single core: on this part the collective subsystem costs ~60us per
execution (a ~44us start barrier plus AllGather trigger latency), while
the entire real workload is one 4MB logits read (~14us) plus small
matmuls.  Any multi-core split must pay the collective tax, so one core
with zero collectives wins by a wide margin.

Scoring: score(p) factors over a prefix/suffix split.  With
w[j,i] = prob[j, target[i]],
    score(p) = A[p[0:4]] + B[p[4:8]],
and argmax over all 40320 perms reduces to
    max_n (A[n] + maxB[complement(subset(n))]).
A and B are materialized directly in [70 subsets (partitions), 24
orderings (free)] layout via a rank factorization:
    A[s, o] = sum_j w[j, elems(s)[rankperm_o(j)]]
            = sum_{(rho,j)} W4[(rho,j), s] * OH24A[(rho,j), o]
where W4[(rho,j), s] = w[j, elems(s)[rho]] is itself one one-hot matmul
from the 64 gathered w values.  Everything downstream (per-subset max,
argmax, complement lookup) runs partition-parallel.

Because subsets are enumerated in lex order, complementation reverses
the order: comp(s) = 69 - s.  The complement lookup maxB[comp] is one
reversal matmul and cstar = 69 - sstar is pure arithmetic.

Index convention (HW-verified in the baseline kernel): r = i*8 + j
encodes the (i,j) pair of w[j, i] at SBUF partition r; j = r % 8,
i = (r % 64) // 8, half = r // 64.
"""

import numpy as np
from itertools import permutations, combinations

import concourse.bacc as bacc
import concourse.mybir as mybir
import concourse.tile as tile
from concourse.bass import IndirectOffsetOnAxis
from concourse.bass_utils import run_bass_kernel_spmd

M = 8
V = 128000
NT = 1680            # ordered distinct 4-tuples of 8 values
NS = 70              # 4-subsets of 8
NCHUNK = 8           # logits DMA/exp chunks
CH = V * M // 128 // NCHUNK   # 1000 cols per chunk on the [128, 8000] view

f32 = mybir.dt.float32
i32 = mybir.dt.int32
u8 = mybir.dt.uint8

AF = mybir.ActivationFunctionType
OP = mybir.AluOpType
AX = mybir.AxisListType

BIG = 1.0e9

# f32 blob column layout (partition dim 128)
C_EYE = 0            # eye(128)                     [128, 128]
C_JSEL = 128         # jsel (S_j row groups)        [128, 8]
C_JH = 136           # jhmask (j,half selector)     [128, 16]
C_OHW1 = 152         # elem(s, rank=half)           [128, 70]
C_OHW2 = 222         # elem(s, rank=2+half)         [128, 70]
C_O24A = 292         # rank one-hot, positions 0-3  [64, 24]
C_O24B = 316         # rank one-hot, positions 4-7  [64, 24]
C_OHC = 340          # reversal (complement) matrix [70, 70]
C_OIDX = 410         # oidx24[s,o] = o              [70, 24]
C_I70 = 434          # iota70 row                   [1, 70]
C_IV64 = 504         # i-index row (r//8)           [1, 64]
C_B4Q = 568          # {0,.25,.5,.75}x2 row         [1, 8]
C_REP8 = 576         # rep8[k, q] = [k == q%8]      [8, 64]
NBLOB = 640


def build_program(dbg=False):
    nc = bacc.Bacc("TRN2", target_bir_lowering=False, debug=False,
                   num_devices=1)

    # ---- I/O ----
    lgf = nc.dram_tensor("lgf", [M, V], f32, kind="ExternalInput").ap()
    lgf_ind = lgf.rearrange("j v -> (j v)").unsqueeze(1)
    blob = nc.dram_tensor("blob", [128, NBLOB], f32, kind="ExternalInput").ap()
    # host-precomputed gather offsets: ioff[r] = (r%8)*V + target[r//8]
    ioff = nc.dram_tensor("ioff", [64, 1], i32, kind="ExternalInput").ap()
    tgtrow = nc.dram_tensor("tgtrow", [1, M], f32, kind="ExternalInput").ap()
    pfv = nc.dram_tensor("pfv", [NT, 4], u8, kind="ExternalInput").ap()
    o_loss = nc.dram_tensor("loss", [1, M], f32, kind="ExternalOutput").ap()
    o_tb = nc.dram_tensor("tbest", [1, M], i32, kind="ExternalOutput").ap()

    with tile.TileContext(nc) as tc:
        with tc.tile_pool(name="sb", bufs=1) as sb, \
             tc.tile_pool(name="ps", bufs=1, space="PSUM") as ps:

            # ---------- stage in ----------
            # sync queue: the big logits chunks, nothing else before them
            L = sb.tile([128, NCHUNK * CH], f32)          # [128, 8000]
            lgr_v = lgf.rearrange("j (s c) -> (j s) c", s=16)
            for k in range(NCHUNK):
                nc.sync.dma_start(L[:, k * CH:(k + 1) * CH],
                                  lgr_v[:, k * CH:(k + 1) * CH])
            # gpsimd queue: offsets first (gates the T gather), then blob
            ioff_t = sb.tile([64, 1], i32)
            nc.gpsimd.dma_start(ioff_t[:], ioff)
            tgf = sb.tile([1, M], f32)
            nc.gpsimd.dma_start(tgf[:], tgtrow)
            B = sb.tile([128, NBLOB], f32)
            nc.gpsimd.dma_start(B[:], blob)

            one1 = B[0:1, C_EYE:C_EYE + 1]
            eye8 = B[0:M, C_EYE:C_EYE + M]
            eye64 = B[0:64, C_EYE:C_EYE + 64]
            eye70 = B[0:NS, C_EYE:C_EYE + NS]
            jsel_v = B[:, C_JSEL:C_JSEL + M]
            jh_v = B[:, C_JH:C_JH + 16]
            ohw1_v = B[:, C_OHW1:C_OHW1 + NS]
            ohw2_v = B[:, C_OHW2:C_OHW2 + NS]
            o24a_v = B[0:64, C_O24A:C_O24A + 24]
            o24b_v = B[0:64, C_O24B:C_O24B + 24]
            ohc_v = B[0:NS, C_OHC:C_OHC + NS]
            oidx_v = B[0:NS, C_OIDX:C_OIDX + 24]
            i70_v = B[0:1, C_I70:C_I70 + NS]
            iv64_v = B[0:1, C_IV64:C_IV64 + 64]
            b4q_v = B[0:1, C_B4Q:C_B4Q + M]
            rep8_v = B[0:8, C_REP8:C_REP8 + 64]

            # T'[j, i] = logits[j, target[i]] at partition r = i*8 + j
            T_p = sb.tile([64, 1], f32)
            nc.gpsimd.indirect_dma_start(
                T_p[:], None, lgf_ind,
                IndirectOffsetOnAxis(ap=ioff_t[:], axis=0))

            # dummy exp on zeroed scratch: pull the ACT exp-table load
            # into the DMA window
            scr8 = sb.tile([1, M], f32)
            nc.vector.memset(scr8[:], 0.0)
            nc.scalar.activation(scr8[:], scr8[:], AF.Exp)

            # ---------- softmax denominators (chunked exp) ----------
            E = sb.tile([128, NCHUNK * CH], f32)
            acc = sb.tile([128, NCHUNK], f32)

            def exp_chunk(k):
                nc.scalar.activation(E[:, k * CH:(k + 1) * CH],
                                     L[:, k * CH:(k + 1) * CH], AF.Exp,
                                     accum_out=acc[:, k:k + 1])

            exp_chunk(0)
            exp_chunk(1)

            Trow_ps = ps.tile([1, 64], f32, tag="t2")
            nc.tensor.matmul(Trow_ps[:], T_p[:], eye64, start=True, stop=True)
            Trow = sb.tile([1, 64], f32)
            nc.vector.tensor_copy(Trow[:], Trow_ps[:])
            # expTrow on the scalar queue after chunks 0-1: T is ready long
            # before chunk 1's exp retires, so the queue never stalls
            expTrow = sb.tile([1, 64], f32)
            nc.scalar.activation(expTrow[:], Trow_ps[:], AF.Exp)

            for k in range(2, NCHUNK):
                exp_chunk(k)

            # ---------- unnormalized W4 table (hidden under the DMA) ----
            # W4raw[(rho,j), s] = exp(T'[j, elems(s)[rho]])
            e128row = sb.tile([1, 128], f32)
            nc.vector.tensor_copy(
                e128row[:].rearrange("p (h r) -> p h r", h=2),
                expTrow[:].unsqueeze(1).to_broadcast((1, 2, 64)))
            eT128_ps = ps.tile([128, 1], f32, tag="t1")
            nc.tensor.matmul(eT128_ps[:], e128row[:], one1,
                             start=True, stop=True)
            eT128 = sb.tile([128, 1], f32)
            nc.vector.tensor_copy(eT128[:], eT128_ps[:])
            WJe = sb.tile([128, 16], f32)
            nc.vector.tensor_scalar(WJe[:], jh_v, eT128[:], None, OP.mult)
            psW1 = ps.tile([16, NS], f32, tag="w1")
            nc.tensor.matmul(psW1[:], WJe[:], ohw1_v, start=True, stop=True)
            psW2 = ps.tile([16, NS], f32, tag="w2")
            nc.tensor.matmul(psW2[:], WJe[:], ohw2_v, start=True, stop=True)
            Wraw = sb.tile([64, NS], f32)
            # rows 16-31 / 48-63 are dead but must be finite: OH24's zero
            # rows would still propagate NaN through the PE accumulate
            nc.vector.memset(Wraw[:], 0.0)
            nc.vector.tensor_copy(Wraw[0:16, :], psW1[:])
            nc.vector.tensor_copy(Wraw[32:48, :], psW2[:])

            # ---------- S_j, 1/S_j, log S_j ----------
            sums = sb.tile([128, 1], f32)
            nc.vector.tensor_reduce(sums[:], acc[:], axis=AX.X, op=OP.add)
            S8_ps = ps.tile([M, 1], f32, tag="t3")
            nc.tensor.matmul(S8_ps[:], jsel_v, sums[:], start=True, stop=True)
            S8sb = sb.tile([M, 1], f32)
            nc.vector.tensor_copy(S8sb[:], S8_ps[:])
            recipS_p = sb.tile([M, 1], f32)
            nc.vector.reciprocal(recipS_p[:], S8sb[:])
            S8row_ps = ps.tile([1, M], f32, tag="t4")
            nc.tensor.matmul(S8row_ps[:], S8sb[:], eye8, start=True, stop=True)
            lseN = sb.tile([1, M], f32)
            nc.scalar.activation(lseN[:], S8row_ps[:], AF.Ln)

            # ---------- normalize W4 and emit A/B in [70, 24] ----------
            rec64_ps = ps.tile([64, 1], f32, tag="t3")
            nc.tensor.matmul(rec64_ps[:], rep8_v, recipS_p[:],
                             start=True, stop=True)
            rec64 = sb.tile([64, 1], f32)
            nc.vector.tensor_copy(rec64[:], rec64_ps[:])
            W4 = sb.tile([64, NS], f32)
            nc.vector.tensor_scalar(W4[:], Wraw[:], rec64[:], None, OP.mult)
            psA = ps.tile([NS, 24], f32, tag="a70")
            nc.tensor.matmul(psA[:], W4[:], o24a_v, start=True, stop=True)
            psB = ps.tile([NS, 24], f32, tag="b70")
            nc.tensor.matmul(psB[:], W4[:], o24b_v, start=True, stop=True)

            A70 = sb.tile([NS, 24], f32)
            nc.vector.tensor_copy(A70[:], psA[:])
            B70 = sb.tile([NS, 24], f32)
            nc.vector.tensor_copy(B70[:], psB[:])

            # ---------- per-subset suffix max + first-argmax ----------
            maxB = sb.tile([NS, 1], f32)
            nc.vector.tensor_reduce(maxB[:], B70[:], axis=AX.X, op=OP.max)
            boE = sb.tile([NS, 24], f32)
            nc.vector.tensor_scalar(boE[:], B70[:], maxB[:], BIG,
                                    OP.is_lt, OP.mult)
            boE2 = sb.tile([NS, 24], f32)
            bo = sb.tile([NS, 1], f32)
            nc.vector.tensor_tensor(boE2[:], boE[:], oidx_v, OP.add)
            nc.vector.tensor_reduce(bo[:], boE2[:], axis=AX.X, op=OP.min)
            # maxBc[s] = maxB[69 - s]
            mBc_ps = ps.tile([NS, 1], f32, tag="t3")
            nc.tensor.matmul(mBc_ps[:], ohc_v, maxB[:], start=True, stop=True)
            maxBc = sb.tile([NS, 1], f32)
            nc.vector.tensor_copy(maxBc[:], mBc_ps[:])

            # ---------- tot = A + maxBc; row maxima and argmaxes ----------
            tot = sb.tile([NS, 24], f32)
            nc.vector.tensor_scalar(tot[:], A70[:], maxBc[:], None, OP.add)
            rmax = sb.tile([NS, 1], f32)
            nc.vector.tensor_reduce(rmax[:], tot[:], axis=AX.X, op=OP.max)
            oE = sb.tile([NS, 24], f32)
            nc.vector.tensor_scalar(oE[:], tot[:], rmax[:], BIG,
                                    OP.is_lt, OP.mult)
            oE2 = sb.tile([NS, 24], f32)
            oarg = sb.tile([NS, 1], f32)
            nc.vector.tensor_tensor(oE2[:], oE[:], oidx_v, OP.add)
            nc.vector.tensor_reduce(oarg[:], oE2[:], axis=AX.X, op=OP.min)

            # cross-partition: transpose the three [70,1] columns to rows
            rmT_ps = ps.tile([1, NS], f32, tag="w1")
            nc.tensor.matmul(rmT_ps[:], rmax[:], eye70, start=True, stop=True)
            oaT_ps = ps.tile([1, NS], f32, tag="t1")
            nc.tensor.matmul(oaT_ps[:], oarg[:], eye70, start=True, stop=True)
            boT_ps = ps.tile([1, NS], f32, tag="t2")
            nc.tensor.matmul(boT_ps[:], bo[:], eye70, start=True, stop=True)
            rmT_sb = sb.tile([1, NS], f32)
            nc.vector.tensor_copy(rmT_sb[:], rmT_ps[:])
            oaT_sb = sb.tile([1, NS], f32)
            nc.vector.tensor_copy(oaT_sb[:], oaT_ps[:])
            boT_sb = sb.tile([1, NS], f32)
            nc.vector.tensor_copy(boT_sb[:], boT_ps[:])
            rmT = rmT_sb[:]
            oaT = oaT_sb[:]
            boT = boT_sb[:]

            # global first-max over subsets
            mfin = sb.tile([1, 1], f32)
            nc.vector.tensor_reduce(mfin[:], rmT, axis=AX.X, op=OP.max)
            es = sb.tile([1, NS], f32)
            nc.vector.tensor_scalar(es[:], rmT, mfin[:], BIG,
                                    OP.is_lt, OP.mult)
            es2 = sb.tile([1, NS], f32)
            sstar = sb.tile([1, 1], f32)
            nc.vector.tensor_tensor(es2[:], es[:], i70_v, OP.add)
            nc.vector.tensor_reduce(sstar[:], es2[:], axis=AX.X, op=OP.min)

            # ostar = oarg[sstar]; cstar = 69 - sstar; bostar = bo[cstar]
            eq1 = sb.tile([1, NS], f32)
            nc.vector.tensor_scalar(eq1[:], i70_v, sstar[:], None, OP.is_equal)
            g1 = sb.tile([1, NS], f32)
            ostar = sb.tile([1, 1], f32)
            nc.vector.tensor_tensor(g1[:], eq1[:], oaT, OP.mult)
            nc.vector.tensor_reduce(ostar[:], g1[:], axis=AX.X, op=OP.add)
            cstar = sb.tile([1, 1], f32)
            nc.vector.tensor_scalar(cstar[:], sstar[:], -1.0, 69.0,
                                    OP.mult, OP.add)  # imm scalars only
            eq2 = sb.tile([1, NS], f32)
            nc.vector.tensor_scalar(eq2[:], i70_v, cstar[:], None, OP.is_equal)
            g3 = sb.tile([1, NS], f32)
            bostar = sb.tile([1, 1], f32)
            nc.vector.tensor_tensor(g3[:], eq2[:], boT, OP.mult)
            nc.vector.tensor_reduce(bostar[:], g3[:], axis=AX.X, op=OP.add)

            # ---------- winning tuple indices -> byte offsets ----------
            # naRAW = sstar*24 + ostar ; offf[0:4] = (b4q + naRAW)*4
            naRAW = sb.tile([1, 1], f32)
            nc.vector.tensor_scalar(naRAW[:], sstar[:], 24.0, None, OP.mult)
            nc.vector.tensor_tensor(naRAW[:], naRAW[:], ostar[:], OP.add)
            nbRAW = sb.tile([1, 1], f32)
            nc.vector.tensor_scalar(nbRAW[:], cstar[:], 24.0, None, OP.mult)
            nc.vector.tensor_tensor(nbRAW[:], nbRAW[:], bostar[:], OP.add)
            offf = sb.tile([1, M], f32)
            nc.vector.tensor_scalar(offf[:, 0:4], b4q_v[:, 0:4], naRAW[:],
                                    None, OP.add)
            nc.vector.tensor_scalar(offf[:, 4:8], b4q_v[:, 4:8], nbRAW[:],
                                    None, OP.add)
            nc.vector.tensor_scalar(offf[:], offf[:], 4.0, None, OP.mult)
            offp_ps = ps.tile([M, 1], f32, tag="t3")
            nc.tensor.matmul(offp_ps[:], offf[:], one1, start=True, stop=True)
            offp = sb.tile([M, 1], i32)
            nc.vector.tensor_copy(offp[:], offp_ps[:])
            pb8 = sb.tile([M, 1], u8)
            nc.gpsimd.indirect_dma_start(
                pb8[:], None, pfv.rearrange("a b -> (a b)").unsqueeze(1),
                IndirectOffsetOnAxis(ap=offp[:], axis=0))
            pb8f = sb.tile([M, 1], f32)
            nc.vector.tensor_copy(pb8f[:], pb8[:])
            pbf_ps = ps.tile([1, M], f32, tag="t4")
            nc.tensor.matmul(pbf_ps[:], pb8f[:], eye8, start=True, stop=True)
            pbf = sb.tile([1, M], f32)
            nc.vector.tensor_copy(pbf[:], pbf_ps[:])

            # ---------- loss and tb for the winning assignment ----------
            # mask[r] = (i(r) == perm_best[j(r)]),  r = i*8 + j
            mask = sb.tile([1, 64], f32)
            nc.vector.tensor_tensor(
                mask[:].rearrange("p (i j) -> p i j", j=8),
                iv64_v.rearrange("p (i j) -> p i j", j=8),
                pbf[:].unsqueeze(1).to_broadcast((1, 8, 8)), OP.is_equal)
            tm = sb.tile([1, 64], f32)
            nc.vector.tensor_tensor(tm[:], mask[:], Trow[:], OP.mult)
            Tb = sb.tile([1, M], f32)
            nc.vector.tensor_reduce(Tb[:],
                                    tm[:].rearrange("p (i j) -> p j i", j=8),
                                    axis=AX.X, op=OP.add)
            lossF = sb.tile([1, M], f32)
            nc.vector.tensor_tensor(lossF[:], lseN[:], Tb[:], OP.subtract)

            tm2 = sb.tile([1, 64], f32)
            nc.vector.tensor_tensor(
                tm2[:].rearrange("p (i j) -> p i j", j=8),
                mask[:].rearrange("p (i j) -> p i j", j=8),
                tgf[:].unsqueeze(2).to_broadcast((1, 8, 8)), OP.mult)
            tbc = sb.tile([1, M], f32)
            nc.vector.tensor_reduce(tbc[:],
                                    tm2[:].rearrange("p (i j) -> p j i", j=8),
                                    axis=AX.X, op=OP.add)
            tbFi = sb.tile([1, M], i32)
            nc.vector.tensor_copy(tbFi[:], tbc[:])

            nc.sync.dma_start(o_loss, lossF[:])
            nc.gpsimd.dma_start(o_tb, tbFi[:])

            if dbg:
                def dump(name, t, shape):
                    o = nc.dram_tensor(name, shape, t.dtype,
                                       kind="ExternalOutput").ap()
                    nc.sync.dma_start(o, t)
                dump("d_sums", sums[:], [128, 1])
                dump("d_Trow", Trow[:], [1, 64])
                dump("d_W4", W4[:], [64, NS])
                dump("d_maxB", maxB[:], [NS, 1])
                dump("d_bo", bo[:], [NS, 1])
                dump("d_maxBc", maxBc[:], [NS, 1])
                dump("d_sstar", sstar[:], [1, 1])
                dump("d_ostar", ostar[:], [1, 1])
                dump("d_cstar", cstar[:], [1, 1])
                dump("d_bostar", bostar[:], [1, 1])
                dump("d_offf", offf[:], [1, M])
                dump("d_pbf", pbf[:], [1, M])

    nc.compile()
    return nc


_NC_CACHE = None


def _get_program():
    global _NC_CACHE
    if _NC_CACHE is None:
        _NC_CACHE = build_program()
    return _NC_CACHE


def _make_tables():
    subsets = list(combinations(range(8), 4))            # 70, lex order
    elems = np.array(subsets, dtype=np.int64)            # [70, 4] sorted
    rp = np.array(list(permutations(range(4))), dtype=np.int64)  # [24, 4]
    tuples = []
    for s in subsets:
        for t in permutations(s):
            tuples.append(t)
    tuples = np.array(tuples, dtype=np.int64)            # [1680, 4]

    r = np.arange(128)
    jr = r % 8
    ir = (r % 64) // 8
    hr = r // 64

    blob = np.zeros((128, NBLOB), dtype=np.float32)
    blob[:, C_EYE:C_EYE + 128] = np.eye(128, dtype=np.float32)
    blob[:, C_JSEL:C_JSEL + M] = (
        np.arange(128)[:, None] // 16 == np.arange(8)[None, :])
    # jhmask[r, q] = [j(r) == q%8] * [half(r) == q//8]
    q = np.arange(16)
    blob[:, C_JH:C_JH + 16] = (
        (jr[:, None] == (q % 8)[None, :]) & (hr[:, None] == (q // 8)[None, :]))
    # OHW1[r, s] = [i(r) == elems(s)[half(r)]]
    blob[:, C_OHW1:C_OHW1 + NS] = (ir[:, None] == elems[:, 0:2].T[hr, :])
    # OHW2[r, s] = [i(r) == elems(s)[2 + half(r)]]
    blob[:, C_OHW2:C_OHW2 + NS] = (ir[:, None] == elems[:, 2:4].T[hr, :])
    # OH24A/B rows: W4 row layout r<16: rho=r//8, j=r%8;
    #               32<=r<48: rho=2+(r-32)//8, j=(r-32)%8; else dead.
    o24a = np.zeros((64, 24), dtype=np.float32)
    o24b = np.zeros((64, 24), dtype=np.float32)
    for rr in range(64):
        if rr < 16:
            rho, j = rr // 8, rr % 8
        elif 32 <= rr < 48:
            rho, j = 2 + (rr - 32) // 8, (rr - 32) % 8
        else:
            continue
        if j <= 3:
            o24a[rr, :] = (rp[:, j] == rho)
        else:
            o24b[rr, :] = (rp[:, j - 4] == rho)
    blob[0:64, C_O24A:C_O24A + 24] = o24a
    blob[0:64, C_O24B:C_O24B + 24] = o24b
    # reversal matrix: ohc[k, s] = [k == 69 - s]
    blob[0:NS, C_OHC:C_OHC + NS] = np.eye(NS, dtype=np.float32)[::-1]
    blob[0:NS, C_OIDX:C_OIDX + 24] = np.arange(24)[None, :]
    blob[0, C_I70:C_I70 + NS] = np.arange(NS)
    blob[0, C_IV64:C_IV64 + 64] = np.arange(64) // 8
    blob[0, C_B4Q:C_B4Q + M] = [0.0, 0.25, 0.5, 0.75] * 2
    blob[0:8, C_REP8:C_REP8 + 64] = (
        np.arange(8)[:, None] == (np.arange(64) % 8)[None, :])

    return blob, tuples.astype(np.uint8)


_TABLES = None


def make_in_maps(logits, target, perms):
    global _TABLES
    if _TABLES is None:
        _TABLES = _make_tables()
    blob, pfv = _TABLES
    logits = np.ascontiguousarray(np.asarray(logits, dtype=np.float32))
    target = np.asarray(target).astype(np.int64).reshape(M)
    r = np.arange(64)
    ioff = ((r % 8) * V + target[r // 8]).astype(np.int32).reshape(64, 1)
    tgtrow = target.astype(np.float32).reshape(1, M)
    return [{"lgf": logits, "blob": blob, "ioff": ioff,
             "tgtrow": tgtrow, "pfv": pfv}]


def run(logits, target, perms, trace=False):
    nc = _get_program()
    in_maps = make_in_maps(logits, target, perms)
    res = run_bass_kernel_spmd(nc, in_maps, core_ids=[0], trace=trace)
    loss = res.results[0]["loss"].reshape(M).astype(np.float32)
    tb = res.results[0]["tbest"].reshape(M).astype(np.int32)
    return loss, tb, res


def kernel(logits, target, perms):
    loss, tb, _ = run(logits, target, perms, trace=False)
    return loss, tb


# revision 15
# speedup vs baseline: 2.2947x; 1.0040x over previous
"""Trainium2 Bass kernel for nn_BertHungarianLoss — single-core version.

Reference computation (M=8, V=128000, P=8!=40320):
    prob  = softmax(logits)                              [M, V]
    score[p] = sum_j prob[j, target[perms[p, j]]]        [P]
    best  = argmax(score)  (first max)
    tb    = target[perms[best]]                          [M]
    loss  = -log_softmax(logits)[j, tb[j]]               [M]
    returns (loss, tb)

Why single core: on this part the collective subsystem costs ~60us per
execution (a ~44us start barrier plus AllGather trigger latency), while
the entire real workload is one 4MB logits read (~14us) plus small
matmuls.  Any multi-core split must pay the collective tax, so one core
with zero collectives wins by a wide margin.

Scoring: score(p) factors over a prefix/suffix split.  With
w[j,i] = prob[j, target[i]],
    score(p) = A[p[0:4]] + B[p[4:8]],
and argmax over all 40320 perms reduces to
    max_n (A[n] + maxB[complement(subset(n))]).
A and B are materialized directly in [70 subsets (partitions), 24
orderings (free)] layout via a rank factorization:
    A[s, o] = sum_j w[j, elems(s)[rankperm_o(j)]]
            = sum_{(rho,j)} W4[(rho,j), s] * OH24A[(rho,j), o]
where W4[(rho,j), s] = w[j, elems(s)[rho]] is itself one one-hot matmul
from the 64 gathered w values.  Everything downstream (per-subset max,
argmax, complement lookup) runs partition-parallel.

Because subsets are enumerated in lex order, complementation reverses
the order: comp(s) = 69 - s.  The complement lookup maxB[comp] is one
reversal matmul and cstar = 69 - sstar is pure arithmetic.

Index convention (HW-verified in the baseline kernel): r = i*8 + j
encodes the (i,j) pair of w[j, i] at SBUF partition r; j = r % 8,
i = (r % 64) // 8, half = r // 64.
"""

import numpy as np
from itertools import permutations, combinations

import concourse.bacc as bacc
import concourse.mybir as mybir
import concourse.tile as tile
from concourse.bass import IndirectOffsetOnAxis
from concourse.bass_utils import run_bass_kernel_spmd

M = 8
V = 128000
NT = 1680            # ordered distinct 4-tuples of 8 values
NS = 70              # 4-subsets of 8
NCHUNK = 8           # logits DMA/exp chunks
CH = V * M // 128 // NCHUNK   # 1000 cols per chunk on the [128, 8000] view

f32 = mybir.dt.float32
i32 = mybir.dt.int32
u8 = mybir.dt.uint8

AF = mybir.ActivationFunctionType
OP = mybir.AluOpType
AX = mybir.AxisListType

BIG = 1.0e9

# f32 blob column layout (partition dim 128)
C_EYE = 0            # eye(128)                     [128, 128]
C_JSEL = 128         # jsel (S_j row groups)        [128, 8]
C_JH = 136           # jhmask (j,half selector)     [128, 16]
C_OHW1 = 152         # elem(s, rank=half)           [128, 70]
C_OHW2 = 222         # elem(s, rank=2+half)         [128, 70]
C_O24A = 292         # rank one-hot, positions 0-3  [64, 24]
C_O24B = 316         # rank one-hot, positions 4-7  [64, 24]
C_OHC = 340          # reversal (complement) matrix [70, 70]
C_OIDX = 410         # oidx24[s,o] = o              [70, 24]
C_I70 = 434          # iota70 row                   [1, 70]
C_IV64 = 504         # i-index row (r//8)           [1, 64]
C_B4Q = 568          # {0,.25,.5,.75}x2 row         [1, 8]
C_REP8 = 576         # rep8[k, q] = [k == q%8]      [8, 64]
NBLOB = 640


def build_program(dbg=False):
    nc = bacc.Bacc("TRN2", target_bir_lowering=False, debug=False,
                   num_devices=1)

    # ---- I/O ----
    lgf = nc.dram_tensor("lgf", [M, V], f32, kind="ExternalInput").ap()
    lgf_ind = lgf.rearrange("j v -> (j v)").unsqueeze(1)
    blob = nc.dram_tensor("blob", [128, NBLOB], f32, kind="ExternalInput").ap()
    # host-precomputed gather offsets: ioff[r] = (r%8)*V + target[r//8]
    ioff = nc.dram_tensor("ioff", [64, 1], i32, kind="ExternalInput").ap()
    tgtrow = nc.dram_tensor("tgtrow", [1, M], f32, kind="ExternalInput").ap()
    pfv = nc.dram_tensor("pfv", [NT, 4], u8, kind="ExternalInput").ap()
    o_loss = nc.dram_tensor("loss", [1, M], f32, kind="ExternalOutput").ap()
    o_tb = nc.dram_tensor("tbest", [1, M], i32, kind="ExternalOutput").ap()

    with tile.TileContext(nc) as tc:
        with tc.tile_pool(name="sb", bufs=1) as sb, \
             tc.tile_pool(name="ps", bufs=1, space="PSUM") as ps:

            # ---------- stage in ----------
            # sync queue: the big logits chunks, nothing else before them
            L = sb.tile([128, NCHUNK * CH], f32)          # [128, 8000]
            lgr_v = lgf.rearrange("j (s c) -> (j s) c", s=16)
            for k in range(NCHUNK):
                nc.sync.dma_start(L[:, k * CH:(k + 1) * CH],
                                  lgr_v[:, k * CH:(k + 1) * CH])
            # gpsimd queue: offsets first (gates the T gather), then blob
            ioff_t = sb.tile([64, 1], i32)
            nc.gpsimd.dma_start(ioff_t[:], ioff)
            tgf = sb.tile([1, M], f32)
            nc.gpsimd.dma_start(tgf[:], tgtrow)
            B = sb.tile([128, NBLOB], f32)
            nc.gpsimd.dma_start(B[:], blob)

            one1 = B[0:1, C_EYE:C_EYE + 1]
            eye8 = B[0:M, C_EYE:C_EYE + M]
            eye64 = B[0:64, C_EYE:C_EYE + 64]
            eye70 = B[0:NS, C_EYE:C_EYE + NS]
            jsel_v = B[:, C_JSEL:C_JSEL + M]
            jh_v = B[:, C_JH:C_JH + 16]
            ohw1_v = B[:, C_OHW1:C_OHW1 + NS]
            ohw2_v = B[:, C_OHW2:C_OHW2 + NS]
            o24a_v = B[0:64, C_O24A:C_O24A + 24]
            o24b_v = B[0:64, C_O24B:C_O24B + 24]
            ohc_v = B[0:NS, C_OHC:C_OHC + NS]
            oidx_v = B[0:NS, C_OIDX:C_OIDX + 24]
            i70_v = B[0:1, C_I70:C_I70 + NS]
            iv64_v = B[0:1, C_IV64:C_IV64 + 64]
            b4q_v = B[0:1, C_B4Q:C_B4Q + M]
            rep8_v = B[0:8, C_REP8:C_REP8 + 64]

            # T'[j, i] = logits[j, target[i]] at partition r = i*8 + j
            T_p = sb.tile([64, 1], f32)
            nc.gpsimd.indirect_dma_start(
                T_p[:], None, lgf_ind,
                IndirectOffsetOnAxis(ap=ioff_t[:], axis=0))

            # dummy exp on zeroed scratch: pull the ACT exp-table load
            # into the DMA window
            scr8 = sb.tile([1, M], f32)
            nc.vector.memset(scr8[:], 0.0)
            nc.scalar.activation(scr8[:], scr8[:], AF.Exp)

            # ---------- softmax denominators (chunked exp) ----------
            E = sb.tile([128, NCHUNK * CH], f32)
            acc = sb.tile([128, NCHUNK], f32)

            def exp_chunk(k):
                nc.scalar.activation(E[:, k * CH:(k + 1) * CH],
                                     L[:, k * CH:(k + 1) * CH], AF.Exp,
                                     accum_out=acc[:, k:k + 1])

            exp_chunk(0)
            exp_chunk(1)

            Trow_ps = ps.tile([1, 64], f32, tag="t2")
            nc.tensor.matmul(Trow_ps[:], T_p[:], eye64, start=True, stop=True)
            Trow = sb.tile([1, 64], f32)
            nc.vector.tensor_copy(Trow[:], Trow_ps[:])
            # expTrow on the scalar queue after chunks 0-1: T is ready long
            # before chunk 1's exp retires, so the queue never stalls
            expTrow = sb.tile([1, 64], f32)
            nc.scalar.activation(expTrow[:], Trow_ps[:], AF.Exp)

            for k in range(2, NCHUNK):
                exp_chunk(k)

            # ---------- unnormalized W4 table (hidden under the DMA) ----
            # W4raw[(rho,j), s] = exp(T'[j, elems(s)[rho]])
            e128row = sb.tile([1, 128], f32)
            nc.vector.tensor_copy(
                e128row[:].rearrange("p (h r) -> p h r", h=2),
                expTrow[:].unsqueeze(1).to_broadcast((1, 2, 64)))
            eT128_ps = ps.tile([128, 1], f32, tag="t1")
            nc.tensor.matmul(eT128_ps[:], e128row[:], one1,
                             start=True, stop=True)
            eT128 = sb.tile([128, 1], f32)
            nc.vector.tensor_copy(eT128[:], eT128_ps[:])
            WJe = sb.tile([128, 16], f32)
            nc.vector.tensor_scalar(WJe[:], jh_v, eT128[:], None, OP.mult)
            psW1 = ps.tile([16, NS], f32, tag="w1")
            nc.tensor.matmul(psW1[:], WJe[:], ohw1_v, start=True, stop=True)
            psW2 = ps.tile([16, NS], f32, tag="w2")
            nc.tensor.matmul(psW2[:], WJe[:], ohw2_v, start=True, stop=True)
            Wraw = sb.tile([64, NS], f32)
            # rows 16-31 / 48-63 are dead but must be finite: OH24's zero
            # rows would still propagate NaN through the PE accumulate
            nc.vector.memset(Wraw[:], 0.0)
            nc.vector.tensor_copy(Wraw[0:16, :], psW1[:])
            nc.vector.tensor_copy(Wraw[32:48, :], psW2[:])

            # ---------- S_j, 1/S_j, log S_j ----------
            sums = sb.tile([128, 1], f32)
            nc.vector.tensor_reduce(sums[:], acc[:], axis=AX.X, op=OP.add)
            S8_ps = ps.tile([M, 1], f32, tag="t3")
            nc.tensor.matmul(S8_ps[:], jsel_v, sums[:], start=True, stop=True)
            S8sb = sb.tile([M, 1], f32)
            nc.vector.tensor_copy(S8sb[:], S8_ps[:])
            recipS_p = sb.tile([M, 1], f32)
            nc.vector.reciprocal(recipS_p[:], S8sb[:])
            S8row_ps = ps.tile([1, M], f32, tag="t4")
            nc.tensor.matmul(S8row_ps[:], S8sb[:], eye8, start=True, stop=True)
            lseN = sb.tile([1, M], f32)
            nc.scalar.activation(lseN[:], S8row_ps[:], AF.Ln)

            # ---------- normalize W4 and emit A/B in [70, 24] ----------
            rec64_ps = ps.tile([64, 1], f32, tag="t3")
            nc.tensor.matmul(rec64_ps[:], rep8_v, recipS_p[:],
                             start=True, stop=True)
            rec64 = sb.tile([64, 1], f32)
            nc.vector.tensor_copy(rec64[:], rec64_ps[:])
            W4 = sb.tile([64, NS], f32)
            nc.vector.tensor_scalar(W4[:], Wraw[:], rec64[:], None, OP.mult)
            psA = ps.tile([NS, 24], f32, tag="a70")
            nc.tensor.matmul(psA[:], W4[:], o24a_v, start=True, stop=True)
            psB = ps.tile([NS, 24], f32, tag="b70")
            nc.tensor.matmul(psB[:], W4[:], o24b_v, start=True, stop=True)

            # ---------- per-subset suffix max + first-argmax ----------
            maxB = sb.tile([NS, 1], f32)
            nc.vector.tensor_reduce(maxB[:], psB[:], axis=AX.X, op=OP.max)
            boE = sb.tile([NS, 24], f32)
            nc.vector.tensor_scalar(boE[:], psB[:], maxB[:], BIG,
                                    OP.is_lt, OP.mult)
            boE2 = sb.tile([NS, 24], f32)
            bo = sb.tile([NS, 1], f32)
            nc.vector.tensor_tensor(boE2[:], boE[:], oidx_v, OP.add)
            nc.vector.tensor_reduce(bo[:], boE2[:], axis=AX.X, op=OP.min)
            # maxBc[s] = maxB[69 - s]
            mBc_ps = ps.tile([NS, 1], f32, tag="t3")
            nc.tensor.matmul(mBc_ps[:], ohc_v, maxB[:], start=True, stop=True)
            maxBc = sb.tile([NS, 1], f32)
            nc.vector.tensor_copy(maxBc[:], mBc_ps[:])

            # ---------- tot = A + maxBc; row maxima and argmaxes ----------
            tot = sb.tile([NS, 24], f32)
            nc.vector.tensor_scalar(tot[:], psA[:], maxBc[:], None, OP.add)
            rmax = sb.tile([NS, 1], f32)
            nc.vector.tensor_reduce(rmax[:], tot[:], axis=AX.X, op=OP.max)
            oE = sb.tile([NS, 24], f32)
            nc.vector.tensor_scalar(oE[:], tot[:], rmax[:], BIG,
                                    OP.is_lt, OP.mult)
            oE2 = sb.tile([NS, 24], f32)
            oarg = sb.tile([NS, 1], f32)
            nc.vector.tensor_tensor(oE2[:], oE[:], oidx_v, OP.add)
            nc.vector.tensor_reduce(oarg[:], oE2[:], axis=AX.X, op=OP.min)

            # cross-partition: transpose the three [70,1] columns to rows
            rmT_ps = ps.tile([1, NS], f32, tag="w1")
            nc.tensor.matmul(rmT_ps[:], rmax[:], eye70, start=True, stop=True)
            oaT_ps = ps.tile([1, NS], f32, tag="t1")
            nc.tensor.matmul(oaT_ps[:], oarg[:], eye70, start=True, stop=True)
            boT_ps = ps.tile([1, NS], f32, tag="t2")
            nc.tensor.matmul(boT_ps[:], bo[:], eye70, start=True, stop=True)
            rmT = rmT_ps[:]
            oaT = oaT_ps[:]
            boT = boT_ps[:]

            # global first-max over subsets
            mfin = sb.tile([1, 1], f32)
            nc.vector.tensor_reduce(mfin[:], rmT, axis=AX.X, op=OP.max)
            es = sb.tile([1, NS], f32)
            nc.vector.tensor_scalar(es[:], rmT, mfin[:], BIG,
                                    OP.is_lt, OP.mult)
            es2 = sb.tile([1, NS], f32)
            sstar = sb.tile([1, 1], f32)
            nc.vector.tensor_tensor(es2[:], es[:], i70_v, OP.add)
            nc.vector.tensor_reduce(sstar[:], es2[:], axis=AX.X, op=OP.min)

            # ostar = oarg[sstar]; cstar = 69 - sstar; bostar = bo[cstar]
            eq1 = sb.tile([1, NS], f32)
            nc.vector.tensor_scalar(eq1[:], i70_v, sstar[:], None, OP.is_equal)
            g1 = sb.tile([1, NS], f32)
            ostar = sb.tile([1, 1], f32)
            nc.vector.tensor_tensor(g1[:], eq1[:], oaT, OP.mult)
            nc.vector.tensor_reduce(ostar[:], g1[:], axis=AX.X, op=OP.add)
            cstar = sb.tile([1, 1], f32)
            nc.vector.tensor_scalar(cstar[:], sstar[:], -1.0, 69.0,
                                    OP.mult, OP.add)  # imm scalars only
            eq2 = sb.tile([1, NS], f32)
            nc.vector.tensor_scalar(eq2[:], i70_v, cstar[:], None, OP.is_equal)
            g3 = sb.tile([1, NS], f32)
            bostar = sb.tile([1, 1], f32)
            nc.vector.tensor_tensor(g3[:], eq2[:], boT, OP.mult)
            nc.vector.tensor_reduce(bostar[:], g3[:], axis=AX.X, op=OP.add)

            # ---------- winning tuple indices -> byte offsets ----------
            # naRAW = sstar*24 + ostar ; offf[0:4] = (b4q + naRAW)*4
            naRAW = sb.tile([1, 1], f32)
            nc.vector.tensor_scalar(naRAW[:], sstar[:], 24.0, None, OP.mult)
            nc.vector.tensor_tensor(naRAW[:], naRAW[:], ostar[:], OP.add)
            nbRAW = sb.tile([1, 1], f32)
            nc.vector.tensor_scalar(nbRAW[:], cstar[:], 24.0, None, OP.mult)
            nc.vector.tensor_tensor(nbRAW[:], nbRAW[:], bostar[:], OP.add)
            offf = sb.tile([1, M], f32)
            nc.vector.tensor_scalar(offf[:, 0:4], b4q_v[:, 0:4], naRAW[:],
                                    None, OP.add)
            nc.vector.tensor_scalar(offf[:, 4:8], b4q_v[:, 4:8], nbRAW[:],
                                    None, OP.add)
            nc.vector.tensor_scalar(offf[:], offf[:], 4.0, None, OP.mult)
            offp_ps = ps.tile([M, 1], f32, tag="t3")
            nc.tensor.matmul(offp_ps[:], offf[:], one1, start=True, stop=True)
            offp = sb.tile([M, 1], i32)
            nc.vector.tensor_copy(offp[:], offp_ps[:])
            pb8 = sb.tile([M, 1], u8)
            nc.gpsimd.indirect_dma_start(
                pb8[:], None, pfv.rearrange("a b -> (a b)").unsqueeze(1),
                IndirectOffsetOnAxis(ap=offp[:], axis=0))
            pb8f = sb.tile([M, 1], f32)
            nc.vector.tensor_copy(pb8f[:], pb8[:])
            pbf_ps = ps.tile([1, M], f32, tag="t4")
            nc.tensor.matmul(pbf_ps[:], pb8f[:], eye8, start=True, stop=True)
            pbf = sb.tile([1, M], f32)
            nc.vector.tensor_copy(pbf[:], pbf_ps[:])

            # ---------- loss and tb for the winning assignment ----------
            # mask[r] = (i(r) == perm_best[j(r)]),  r = i*8 + j
            mask = sb.tile([1, 64], f32)
            nc.vector.tensor_tensor(
                mask[:].rearrange("p (i j) -> p i j", j=8),
                iv64_v.rearrange("p (i j) -> p i j", j=8),
                pbf[:].unsqueeze(1).to_broadcast((1, 8, 8)), OP.is_equal)
            tm = sb.tile([1, 64], f32)
            nc.vector.tensor_tensor(tm[:], mask[:], Trow[:], OP.mult)
            Tb = sb.tile([1, M], f32)
            nc.vector.tensor_reduce(Tb[:],
                                    tm[:].rearrange("p (i j) -> p j i", j=8),
                                    axis=AX.X, op=OP.add)
            lossF = sb.tile([1, M], f32)
            nc.vector.tensor_tensor(lossF[:], lseN[:], Tb[:], OP.subtract)

            tm2 = sb.tile([1, 64], f32)
            nc.vector.tensor_tensor(
                tm2[:].rearrange("p (i j) -> p i j", j=8),
                mask[:].rearrange("p (i j) -> p i j", j=8),
                tgf[:].unsqueeze(2).to_broadcast((1, 8, 8)), OP.mult)
            tbc = sb.tile([1, M], f32)
            nc.vector.tensor_reduce(tbc[:],
                                    tm2[:].rearrange("p (i j) -> p j i", j=8),
                                    axis=AX.X, op=OP.add)
            tbFi = sb.tile([1, M], i32)
            nc.vector.tensor_copy(tbFi[:], tbc[:])

            nc.sync.dma_start(o_loss, lossF[:])
            nc.gpsimd.dma_start(o_tb, tbFi[:])

            if dbg:
                def dump(name, t, shape):
                    o = nc.dram_tensor(name, shape, t.dtype,
                                       kind="ExternalOutput").ap()
                    nc.sync.dma_start(o, t)
                dump("d_sums", sums[:], [128, 1])
                dump("d_Trow", Trow[:], [1, 64])
                dump("d_W4", W4[:], [64, NS])
                dump("d_maxB", maxB[:], [NS, 1])
                dump("d_bo", bo[:], [NS, 1])
                dump("d_maxBc", maxBc[:], [NS, 1])
                dump("d_sstar", sstar[:], [1, 1])
                dump("d_ostar", ostar[:], [1, 1])
                dump("d_cstar", cstar[:], [1, 1])
                dump("d_bostar", bostar[:], [1, 1])
                dump("d_offf", offf[:], [1, M])
                dump("d_pbf", pbf[:], [1, M])

    nc.compile()
    return nc


_NC_CACHE = None


def _get_program():
    global _NC_CACHE
    if _NC_CACHE is None:
        _NC_CACHE = build_program()
    return _NC_CACHE


def _make_tables():
    subsets = list(combinations(range(8), 4))            # 70, lex order
    elems = np.array(subsets, dtype=np.int64)            # [70, 4] sorted
    rp = np.array(list(permutations(range(4))), dtype=np.int64)  # [24, 4]
    tuples = []
    for s in subsets:
        for t in permutations(s):
            tuples.append(t)
    tuples = np.array(tuples, dtype=np.int64)            # [1680, 4]

    r = np.arange(128)
    jr = r % 8
    ir = (r % 64) // 8
    hr = r // 64

    blob = np.zeros((128, NBLOB), dtype=np.float32)
    blob[:, C_EYE:C_EYE + 128] = np.eye(128, dtype=np.float32)
    blob[:, C_JSEL:C_JSEL + M] = (
        np.arange(128)[:, None] // 16 == np.arange(8)[None, :])
    # jhmask[r, q] = [j(r) == q%8] * [half(r) == q//8]
    q = np.arange(16)
    blob[:, C_JH:C_JH + 16] = (
        (jr[:, None] == (q % 8)[None, :]) & (hr[:, None] == (q // 8)[None, :]))
    # OHW1[r, s] = [i(r) == elems(s)[half(r)]]
    blob[:, C_OHW1:C_OHW1 + NS] = (ir[:, None] == elems[:, 0:2].T[hr, :])
    # OHW2[r, s] = [i(r) == elems(s)[2 + half(r)]]
    blob[:, C_OHW2:C_OHW2 + NS] = (ir[:, None] == elems[:, 2:4].T[hr, :])
    # OH24A/B rows: W4 row layout r<16: rho=r//8, j=r%8;
    #               32<=r<48: rho=2+(r-32)//8, j=(r-32)%8; else dead.
    o24a = np.zeros((64, 24), dtype=np.float32)
    o24b = np.zeros((64, 24), dtype=np.float32)
    for rr in range(64):
        if rr < 16:
            rho, j = rr // 8, rr % 8
        elif 32 <= rr < 48:
            rho, j = 2 + (rr - 32) // 8, (rr - 32) % 8
        else:
            continue
        if j <= 3:
            o24a[rr, :] = (rp[:, j] == rho)
        else:
            o24b[rr, :] = (rp[:, j - 4] == rho)
    blob[0:64, C_O24A:C_O24A + 24] = o24a
    blob[0:64, C_O24B:C_O24B + 24] = o24b
    # reversal matrix: ohc[k, s] = [k == 69 - s]
    blob[0:NS, C_OHC:C_OHC + NS] = np.eye(NS, dtype=np.float32)[::-1]
    blob[0:NS, C_OIDX:C_OIDX + 24] = np.arange(24)[None, :]
    blob[0, C_I70:C_I70 + NS] = np.arange(NS)
    blob[0, C_IV64:C_IV64 + 64] = np.arange(64) // 8
    blob[0, C_B4Q:C_B4Q + M] = [0.0, 0.25, 0.5, 0.75] * 2
    blob[0:8, C_REP8:C_REP8 + 64] = (
        np.arange(8)[:, None] == (np.arange(64) % 8)[None, :])

    return blob, tuples.astype(np.uint8)


_TABLES = None


def make_in_maps(logits, target, perms):
    global _TABLES
    if _TABLES is None:
        _TABLES = _make_tables()
    blob, pfv = _TABLES
    logits = np.ascontiguousarray(np.asarray(logits, dtype=np.float32))
    target = np.asarray(target).astype(np.int64).reshape(M)
    r = np.arange(64)
    ioff = ((r % 8) * V + target[r // 8]).astype(np.int32).reshape(64, 1)
    tgtrow = target.astype(np.float32).reshape(1, M)
    return [{"lgf": logits, "blob": blob, "ioff": ioff,
             "tgtrow": tgtrow, "pfv": pfv}]


def run(logits, target, perms, trace=False):
    nc = _get_program()
    in_maps = make_in_maps(logits, target, perms)
    res = run_bass_kernel_spmd(nc, in_maps, core_ids=[0], trace=trace)
    loss = res.results[0]["loss"].reshape(M).astype(np.float32)
    tb = res.results[0]["tbest"].reshape(M).astype(np.int32)
    return loss, tb, res


def kernel(logits, target, perms):
    loss, tb, _ = run(logits, target, perms, trace=False)
    return loss, tb


# revision 16
# speedup vs baseline: 2.3010x; 1.0027x over previous
"""Trainium2 Bass kernel for nn_BertHungarianLoss — single-core version.

Reference computation (M=8, V=128000, P=8!=40320):
    prob  = softmax(logits)                              [M, V]
    score[p] = sum_j prob[j, target[perms[p, j]]]        [P]
    best  = argmax(score)  (first max)
    tb    = target[perms[best]]                          [M]
    loss  = -log_softmax(logits)[j, tb[j]]               [M]
    returns (loss, tb)

Why single core: on this part the collective subsystem costs ~60us per
execution (a ~44us start barrier plus AllGather trigger latency), while
the entire real workload is one 4MB logits read (~14us) plus small
matmuls.  Any multi-core split must pay the collective tax, so one core
with zero collectives wins by a wide margin.

Scoring: score(p) factors over a prefix/suffix split.  With
w[j,i] = prob[j, target[i]],
    score(p) = A[p[0:4]] + B[p[4:8]],
and argmax over all 40320 perms reduces to
    max_n (A[n] + maxB[complement(subset(n))]).
A and B are materialized directly in [70 subsets (partitions), 24
orderings (free)] layout via a rank factorization:
    A[s, o] = sum_j w[j, elems(s)[rankperm_o(j)]]
            = sum_{(rho,j)} W4[(rho,j), s] * OH24A[(rho,j), o]
where W4[(rho,j), s] = w[j, elems(s)[rho]] is itself one one-hot matmul
from the 64 gathered w values.  Everything downstream (per-subset max,
argmax, complement lookup) runs partition-parallel.

Because subsets are enumerated in lex order, complementation reverses
the order: comp(s) = 69 - s.  The complement lookup maxB[comp] is one
reversal matmul and cstar = 69 - sstar is pure arithmetic.

Index convention (HW-verified in the baseline kernel): r = i*8 + j
encodes the (i,j) pair of w[j, i] at SBUF partition r; j = r % 8,
i = (r % 64) // 8, half = r // 64.
"""

import numpy as np
from itertools import permutations, combinations

import concourse.bacc as bacc
import concourse.mybir as mybir
import concourse.tile as tile
from concourse.bass import IndirectOffsetOnAxis
from concourse.bass_utils import run_bass_kernel_spmd

M = 8
V = 128000
NT = 1680            # ordered distinct 4-tuples of 8 values
NS = 70              # 4-subsets of 8
NCHUNK = 8           # logits DMA/exp chunks
CH = V * M // 128 // NCHUNK   # 1000 cols per chunk on the [128, 8000] view

f32 = mybir.dt.float32
i32 = mybir.dt.int32
u8 = mybir.dt.uint8

AF = mybir.ActivationFunctionType
OP = mybir.AluOpType
AX = mybir.AxisListType

BIG = 1.0e9

# f32 blob column layout (partition dim 128)
C_EYE = 0            # eye(128)                     [128, 128]
C_JSEL = 128         # jsel (S_j row groups)        [128, 8]
C_JH = 136           # jhmask (j,half selector)     [128, 16]
C_OHW1 = 152         # elem(s, rank=half)           [128, 70]
C_OHW2 = 222         # elem(s, rank=2+half)         [128, 70]
C_O24A = 292         # rank one-hot, positions 0-3  [64, 24]
C_O24B = 316         # rank one-hot, positions 4-7  [64, 24]
C_OHC = 340          # reversal (complement) matrix [70, 70]
C_OIDX = 410         # oidx24[s,o] = o              [70, 24]
C_I70 = 434          # iota70 row                   [1, 70]
C_IV64 = 504         # i-index row (r//8)           [1, 64]
C_B4Q = 568          # {0,.25,.5,.75}x2 row         [1, 8]
C_REP8 = 576         # rep8[k, q] = [k == q%8]      [8, 64]
NBLOB = 640


def build_program(dbg=False):
    nc = bacc.Bacc("TRN2", target_bir_lowering=False, debug=False,
                   num_devices=1)

    # ---- I/O ----
    lgf = nc.dram_tensor("lgf", [M, V], f32, kind="ExternalInput").ap()
    lgf_ind = lgf.rearrange("j v -> (j v)").unsqueeze(1)
    blob = nc.dram_tensor("blob", [128, NBLOB], f32, kind="ExternalInput").ap()
    # host-precomputed gather offsets: ioff[r] = (r%8)*V + target[r//8]
    ioff = nc.dram_tensor("ioff", [64, 1], i32, kind="ExternalInput").ap()
    tgtrow = nc.dram_tensor("tgtrow", [1, M], f32, kind="ExternalInput").ap()
    pfv = nc.dram_tensor("pfv", [NT, 4], u8, kind="ExternalInput").ap()
    o_loss = nc.dram_tensor("loss", [1, M], f32, kind="ExternalOutput").ap()
    o_tb = nc.dram_tensor("tbest", [1, M], i32, kind="ExternalOutput").ap()

    with tile.TileContext(nc) as tc:
        with tc.tile_pool(name="sb", bufs=1) as sb, \
             tc.tile_pool(name="ps", bufs=1, space="PSUM") as ps:

            # ---------- stage in ----------
            # sync queue: the big logits chunks, nothing else before them
            L = sb.tile([128, NCHUNK * CH], f32)          # [128, 8000]
            lgr_v = lgf.rearrange("j (s c) -> (j s) c", s=16)
            for k in range(NCHUNK):
                nc.sync.dma_start(L[:, k * CH:(k + 1) * CH],
                                  lgr_v[:, k * CH:(k + 1) * CH])
            # gpsimd queue: offsets first (gates the T gather), then blob
            ioff_t = sb.tile([64, 1], i32)
            nc.gpsimd.dma_start(ioff_t[:], ioff)
            tgf = sb.tile([1, M], f32)
            nc.gpsimd.dma_start(tgf[:], tgtrow)
            B = sb.tile([128, NBLOB], f32)
            nc.gpsimd.dma_start(B[:], blob)

            one1 = B[0:1, C_EYE:C_EYE + 1]
            eye8 = B[0:M, C_EYE:C_EYE + M]
            eye64 = B[0:64, C_EYE:C_EYE + 64]
            eye70 = B[0:NS, C_EYE:C_EYE + NS]
            jsel_v = B[:, C_JSEL:C_JSEL + M]
            jh_v = B[:, C_JH:C_JH + 16]
            ohw1_v = B[:, C_OHW1:C_OHW1 + NS]
            ohw2_v = B[:, C_OHW2:C_OHW2 + NS]
            o24a_v = B[0:64, C_O24A:C_O24A + 24]
            o24b_v = B[0:64, C_O24B:C_O24B + 24]
            ohc_v = B[0:NS, C_OHC:C_OHC + NS]
            oidx_v = B[0:NS, C_OIDX:C_OIDX + 24]
            i70_v = B[0:1, C_I70:C_I70 + NS]
            iv64_v = B[0:1, C_IV64:C_IV64 + 64]
            b4q_v = B[0:1, C_B4Q:C_B4Q + M]
            rep8_v = B[0:8, C_REP8:C_REP8 + 64]

            # T'[j, i] = logits[j, target[i]] at partition r = i*8 + j
            T_p = sb.tile([64, 1], f32)
            nc.gpsimd.indirect_dma_start(
                T_p[:], None, lgf_ind,
                IndirectOffsetOnAxis(ap=ioff_t[:], axis=0))

            # dummy exp on zeroed scratch: pull the ACT exp-table load
            # into the DMA window
            scr8 = sb.tile([1, M], f32)
            nc.vector.memset(scr8[:], 0.0)
            nc.scalar.activation(scr8[:], scr8[:], AF.Exp)

            # ---------- softmax denominators (chunked exp) ----------
            E = sb.tile([128, NCHUNK * CH], f32)
            acc = sb.tile([128, NCHUNK], f32)

            def exp_chunk(k):
                nc.scalar.activation(E[:, k * CH:(k + 1) * CH],
                                     L[:, k * CH:(k + 1) * CH], AF.Exp,
                                     accum_out=acc[:, k:k + 1])

            exp_chunk(0)
            exp_chunk(1)

            Trow_ps = ps.tile([1, 64], f32, tag="t2")
            nc.tensor.matmul(Trow_ps[:], T_p[:], eye64, start=True, stop=True)
            Trow = sb.tile([1, 64], f32)
            nc.vector.tensor_copy(Trow[:], Trow_ps[:])
            # expTrow on the scalar queue after chunks 0-1: T is ready long
            # before chunk 1's exp retires, so the queue never stalls
            expTrow = sb.tile([1, 64], f32)
            nc.scalar.activation(expTrow[:], Trow_ps[:], AF.Exp)

            for k in range(2, NCHUNK):
                exp_chunk(k)

            # ---------- unnormalized W4 table (hidden under the DMA) ----
            # W4raw[(rho,j), s] = exp(T'[j, elems(s)[rho]])
            e128row = sb.tile([1, 128], f32)
            nc.vector.tensor_copy(
                e128row[:].rearrange("p (h r) -> p h r", h=2),
                expTrow[:].unsqueeze(1).to_broadcast((1, 2, 64)))
            eT128_ps = ps.tile([128, 1], f32, tag="t1")
            nc.tensor.matmul(eT128_ps[:], e128row[:], one1,
                             start=True, stop=True)
            eT128 = sb.tile([128, 1], f32)
            nc.vector.tensor_copy(eT128[:], eT128_ps[:])
            WJe = sb.tile([128, 16], f32)
            nc.vector.tensor_scalar(WJe[:], jh_v, eT128[:], None, OP.mult)
            psW1 = ps.tile([16, NS], f32, tag="w1")
            nc.tensor.matmul(psW1[:], WJe[:], ohw1_v, start=True, stop=True)
            psW2 = ps.tile([16, NS], f32, tag="w2")
            nc.tensor.matmul(psW2[:], WJe[:], ohw2_v, start=True, stop=True)
            Wraw = sb.tile([64, NS], f32)
            # rows 16-31 / 48-63 are dead but must be finite: OH24's zero
            # rows would still propagate NaN through the PE accumulate
            nc.vector.memset(Wraw[:], 0.0)
            nc.vector.tensor_copy(Wraw[0:16, :], psW1[:])
            nc.vector.tensor_copy(Wraw[32:48, :], psW2[:])

            # ---------- S_j, 1/S_j, log S_j ----------
            sums = sb.tile([128, 1], f32)
            nc.vector.tensor_reduce(sums[:], acc[:], axis=AX.X, op=OP.add)
            S8_ps = ps.tile([M, 1], f32, tag="t3")
            nc.tensor.matmul(S8_ps[:], jsel_v, sums[:], start=True, stop=True)
            S8sb = sb.tile([M, 1], f32)
            nc.vector.tensor_copy(S8sb[:], S8_ps[:])
            recipS_p = sb.tile([M, 1], f32)
            nc.vector.reciprocal(recipS_p[:], S8sb[:])
            S8row_ps = ps.tile([1, M], f32, tag="t4")
            nc.tensor.matmul(S8row_ps[:], S8sb[:], eye8, start=True, stop=True)
            lseN = sb.tile([1, M], f32)
            nc.scalar.activation(lseN[:], S8row_ps[:], AF.Ln)

            # ---------- normalize W4 and emit A/B in [70, 24] ----------
            rec64_ps = ps.tile([64, 1], f32, tag="t3")
            nc.tensor.matmul(rec64_ps[:], rep8_v, recipS_p[:],
                             start=True, stop=True)
            rec64 = sb.tile([64, 1], f32)
            nc.vector.tensor_copy(rec64[:], rec64_ps[:])
            W4 = sb.tile([64, NS], f32)
            nc.vector.tensor_scalar(W4[:], Wraw[:], rec64[:], None, OP.mult)
            psA = ps.tile([NS, 24], f32, tag="a70")
            nc.tensor.matmul(psA[:], W4[:], o24a_v, start=True, stop=True)
            psB = ps.tile([NS, 24], f32, tag="b70")
            nc.tensor.matmul(psB[:], W4[:], o24b_v, start=True, stop=True)

            # ---------- per-subset suffix max + first-argmax ----------
            maxB = sb.tile([NS, 1], f32)
            nc.vector.tensor_reduce(maxB[:], psB[:], axis=AX.X, op=OP.max)
            boE = sb.tile([NS, 24], f32)
            nc.vector.tensor_scalar(boE[:], psB[:], maxB[:], BIG,
                                    OP.is_lt, OP.mult)
            boE2 = sb.tile([NS, 24], f32)
            bo = sb.tile([NS, 1], f32)
            nc.vector.tensor_tensor(boE2[:], boE[:], oidx_v, OP.add)
            nc.vector.tensor_reduce(bo[:], boE2[:], axis=AX.X, op=OP.min)
            # maxBc[s] = maxB[69 - s]
            mBc_ps = ps.tile([NS, 1], f32, tag="t3")
            nc.tensor.matmul(mBc_ps[:], ohc_v, maxB[:], start=True, stop=True)
            maxBc = sb.tile([NS, 1], f32)
            nc.vector.tensor_copy(maxBc[:], mBc_ps[:])

            # ---------- tot = A + maxBc; row maxima and argmaxes ----------
            tot = sb.tile([NS, 24], f32)
            nc.vector.tensor_scalar(tot[:], psA[:], maxBc[:], None, OP.add)
            rmax = sb.tile([NS, 1], f32)
            nc.vector.tensor_reduce(rmax[:], tot[:], axis=AX.X, op=OP.max)
            oE = sb.tile([NS, 24], f32)
            nc.vector.tensor_scalar(oE[:], tot[:], rmax[:], BIG,
                                    OP.is_lt, OP.mult)
            oE2 = sb.tile([NS, 24], f32)
            oarg = sb.tile([NS, 1], f32)
            nc.vector.tensor_tensor(oE2[:], oE[:], oidx_v, OP.add)
            nc.vector.tensor_reduce(oarg[:], oE2[:], axis=AX.X, op=OP.min)

            # cross-partition: transpose the three [70,1] columns to rows
            rmT_ps = ps.tile([1, NS], f32, tag="w1")
            nc.tensor.matmul(rmT_ps[:], rmax[:], eye70, start=True, stop=True)
            oaT_ps = ps.tile([1, NS], f32, tag="t1")
            nc.tensor.matmul(oaT_ps[:], oarg[:], eye70, start=True, stop=True)
            boT_ps = ps.tile([1, NS], f32, tag="t2")
            nc.tensor.matmul(boT_ps[:], bo[:], eye70, start=True, stop=True)
            rmT = rmT_ps[:]
            oaT = oaT_ps[:]
            boT = boT_ps[:]

            # global first-max over subsets
            mfin = sb.tile([1, 1], f32)
            nc.vector.tensor_reduce(mfin[:], rmT, axis=AX.X, op=OP.max)
            es = sb.tile([1, NS], f32)
            nc.vector.tensor_scalar(es[:], rmT, mfin[:], BIG,
                                    OP.is_lt, OP.mult)
            es2 = sb.tile([1, NS], f32)
            sstar = sb.tile([1, 1], f32)
            nc.vector.tensor_tensor(es2[:], es[:], i70_v, OP.add)
            nc.vector.tensor_reduce(sstar[:], es2[:], axis=AX.X, op=OP.min)

            # ostar = oarg[sstar]; cstar = 69 - sstar; bostar = bo[cstar]
            eq1 = sb.tile([1, NS], f32)
            nc.vector.tensor_scalar(eq1[:], i70_v, sstar[:], None, OP.is_equal)
            g1 = sb.tile([1, NS], f32)
            ostar = sb.tile([1, 1], f32)
            nc.vector.tensor_tensor(g1[:], eq1[:], oaT, OP.mult)
            nc.vector.tensor_reduce(ostar[:], g1[:], axis=AX.X, op=OP.add)
            cstar = sb.tile([1, 1], f32)
            nc.vector.tensor_scalar(cstar[:], sstar[:], -1.0, 69.0,
                                    OP.mult, OP.add)  # imm scalars only
            eq2 = sb.tile([1, NS], f32)
            nc.vector.tensor_scalar(eq2[:], i70_v, cstar[:], None, OP.is_equal)
            g3 = sb.tile([1, NS], f32)
            bostar = sb.tile([1, 1], f32)
            nc.vector.tensor_tensor(g3[:], eq2[:], boT, OP.mult)
            nc.vector.tensor_reduce(bostar[:], g3[:], axis=AX.X, op=OP.add)

            # ---------- winning tuple indices -> byte offsets ----------
            # naRAW = sstar*24 + ostar ; offf[0:4] = (b4q + naRAW)*4
            naRAW = sb.tile([1, 1], f32)
            nc.vector.tensor_scalar(naRAW[:], sstar[:], 24.0, ostar[:],
                                    OP.mult, OP.add)
            nbRAW = sb.tile([1, 1], f32)
            nc.vector.tensor_scalar(nbRAW[:], cstar[:], 24.0, bostar[:],
                                    OP.mult, OP.add)
            offf = sb.tile([1, M], f32)
            nc.vector.tensor_scalar(offf[:, 0:4], b4q_v[:, 0:4], naRAW[:],
                                    4.0, OP.add, OP.mult)
            nc.vector.tensor_scalar(offf[:, 4:8], b4q_v[:, 4:8], nbRAW[:],
                                    4.0, OP.add, OP.mult)
            offp_ps = ps.tile([M, 1], f32, tag="t3")
            nc.tensor.matmul(offp_ps[:], offf[:], one1, start=True, stop=True)
            offp = sb.tile([M, 1], i32)
            nc.vector.tensor_copy(offp[:], offp_ps[:])
            pb8 = sb.tile([M, 1], u8)
            nc.gpsimd.indirect_dma_start(
                pb8[:], None, pfv.rearrange("a b -> (a b)").unsqueeze(1),
                IndirectOffsetOnAxis(ap=offp[:], axis=0))
            pb8f = sb.tile([M, 1], f32)
            nc.vector.tensor_copy(pb8f[:], pb8[:])
            pbf_ps = ps.tile([1, M], f32, tag="t4")
            nc.tensor.matmul(pbf_ps[:], pb8f[:], eye8, start=True, stop=True)
            pbf = sb.tile([1, M], f32)
            nc.vector.tensor_copy(pbf[:], pbf_ps[:])

            # ---------- loss and tb for the winning assignment ----------
            # mask[r] = (i(r) == perm_best[j(r)]),  r = i*8 + j
            mask = sb.tile([1, 64], f32)
            nc.vector.tensor_tensor(
                mask[:].rearrange("p (i j) -> p i j", j=8),
                iv64_v.rearrange("p (i j) -> p i j", j=8),
                pbf[:].unsqueeze(1).to_broadcast((1, 8, 8)), OP.is_equal)
            tm = sb.tile([1, 64], f32)
            nc.vector.tensor_tensor(tm[:], mask[:], Trow[:], OP.mult)
            Tb = sb.tile([1, M], f32)
            nc.vector.tensor_reduce(Tb[:],
                                    tm[:].rearrange("p (i j) -> p j i", j=8),
                                    axis=AX.X, op=OP.add)
            lossF = sb.tile([1, M], f32)
            nc.vector.tensor_tensor(lossF[:], lseN[:], Tb[:], OP.subtract)

            tm2 = sb.tile([1, 64], f32)
            nc.vector.tensor_tensor(
                tm2[:].rearrange("p (i j) -> p i j", j=8),
                mask[:].rearrange("p (i j) -> p i j", j=8),
                tgf[:].unsqueeze(2).to_broadcast((1, 8, 8)), OP.mult)
            tbc = sb.tile([1, M], f32)
            nc.vector.tensor_reduce(tbc[:],
                                    tm2[:].rearrange("p (i j) -> p j i", j=8),
                                    axis=AX.X, op=OP.add)
            tbFi = sb.tile([1, M], i32)
            nc.vector.tensor_copy(tbFi[:], tbc[:])

            nc.sync.dma_start(o_loss, lossF[:])
            nc.gpsimd.dma_start(o_tb, tbFi[:])

            if dbg:
                def dump(name, t, shape):
                    o = nc.dram_tensor(name, shape, t.dtype,
                                       kind="ExternalOutput").ap()
                    nc.sync.dma_start(o, t)
                dump("d_sums", sums[:], [128, 1])
                dump("d_Trow", Trow[:], [1, 64])
                dump("d_W4", W4[:], [64, NS])
                dump("d_maxB", maxB[:], [NS, 1])
                dump("d_bo", bo[:], [NS, 1])
                dump("d_maxBc", maxBc[:], [NS, 1])
                dump("d_sstar", sstar[:], [1, 1])
                dump("d_ostar", ostar[:], [1, 1])
                dump("d_cstar", cstar[:], [1, 1])
                dump("d_bostar", bostar[:], [1, 1])
                dump("d_offf", offf[:], [1, M])
                dump("d_pbf", pbf[:], [1, M])

    nc.compile()
    return nc


_NC_CACHE = None


def _get_program():
    global _NC_CACHE
    if _NC_CACHE is None:
        _NC_CACHE = build_program()
    return _NC_CACHE


def _make_tables():
    subsets = list(combinations(range(8), 4))            # 70, lex order
    elems = np.array(subsets, dtype=np.int64)            # [70, 4] sorted
    rp = np.array(list(permutations(range(4))), dtype=np.int64)  # [24, 4]
    tuples = []
    for s in subsets:
        for t in permutations(s):
            tuples.append(t)
    tuples = np.array(tuples, dtype=np.int64)            # [1680, 4]

    r = np.arange(128)
    jr = r % 8
    ir = (r % 64) // 8
    hr = r // 64

    blob = np.zeros((128, NBLOB), dtype=np.float32)
    blob[:, C_EYE:C_EYE + 128] = np.eye(128, dtype=np.float32)
    blob[:, C_JSEL:C_JSEL + M] = (
        np.arange(128)[:, None] // 16 == np.arange(8)[None, :])
    # jhmask[r, q] = [j(r) == q%8] * [half(r) == q//8]
    q = np.arange(16)
    blob[:, C_JH:C_JH + 16] = (
        (jr[:, None] == (q % 8)[None, :]) & (hr[:, None] == (q // 8)[None, :]))
    # OHW1[r, s] = [i(r) == elems(s)[half(r)]]
    blob[:, C_OHW1:C_OHW1 + NS] = (ir[:, None] == elems[:, 0:2].T[hr, :])
    # OHW2[r, s] = [i(r) == elems(s)[2 + half(r)]]
    blob[:, C_OHW2:C_OHW2 + NS] = (ir[:, None] == elems[:, 2:4].T[hr, :])
    # OH24A/B rows: W4 row layout r<16: rho=r//8, j=r%8;
    #               32<=r<48: rho=2+(r-32)//8, j=(r-32)%8; else dead.
    o24a = np.zeros((64, 24), dtype=np.float32)
    o24b = np.zeros((64, 24), dtype=np.float32)
    for rr in range(64):
        if rr < 16:
            rho, j = rr // 8, rr % 8
        elif 32 <= rr < 48:
            rho, j = 2 + (rr - 32) // 8, (rr - 32) % 8
        else:
            continue
        if j <= 3:
            o24a[rr, :] = (rp[:, j] == rho)
        else:
            o24b[rr, :] = (rp[:, j - 4] == rho)
    blob[0:64, C_O24A:C_O24A + 24] = o24a
    blob[0:64, C_O24B:C_O24B + 24] = o24b
    # reversal matrix: ohc[k, s] = [k == 69 - s]
    blob[0:NS, C_OHC:C_OHC + NS] = np.eye(NS, dtype=np.float32)[::-1]
    blob[0:NS, C_OIDX:C_OIDX + 24] = np.arange(24)[None, :]
    blob[0, C_I70:C_I70 + NS] = np.arange(NS)
    blob[0, C_IV64:C_IV64 + 64] = np.arange(64) // 8
    blob[0, C_B4Q:C_B4Q + M] = [0.0, 0.25, 0.5, 0.75] * 2
    blob[0:8, C_REP8:C_REP8 + 64] = (
        np.arange(8)[:, None] == (np.arange(64) % 8)[None, :])

    return blob, tuples.astype(np.uint8)


_TABLES = None


def make_in_maps(logits, target, perms):
    global _TABLES
    if _TABLES is None:
        _TABLES = _make_tables()
    blob, pfv = _TABLES
    logits = np.ascontiguousarray(np.asarray(logits, dtype=np.float32))
    target = np.asarray(target).astype(np.int64).reshape(M)
    r = np.arange(64)
    ioff = ((r % 8) * V + target[r // 8]).astype(np.int32).reshape(64, 1)
    tgtrow = target.astype(np.float32).reshape(1, M)
    return [{"lgf": logits, "blob": blob, "ioff": ioff,
             "tgtrow": tgtrow, "pfv": pfv}]


def run(logits, target, perms, trace=False):
    nc = _get_program()
    in_maps = make_in_maps(logits, target, perms)
    res = run_bass_kernel_spmd(nc, in_maps, core_ids=[0], trace=trace)
    loss = res.results[0]["loss"].reshape(M).astype(np.float32)
    tb = res.results[0]["tbest"].reshape(M).astype(np.int32)
    return loss, tb, res


def kernel(logits, target, perms):
    loss, tb, _ = run(logits, target, perms, trace=False)
    return loss, tb


# revision 17
# speedup vs baseline: 2.3091x; 1.0035x over previous
"""Trainium2 Bass kernel for nn_BertHungarianLoss — single-core version.

Reference computation (M=8, V=128000, P=8!=40320):
    prob  = softmax(logits)                              [M, V]
    score[p] = sum_j prob[j, target[perms[p, j]]]        [P]
    best  = argmax(score)  (first max)
    tb    = target[perms[best]]                          [M]
    loss  = -log_softmax(logits)[j, tb[j]]               [M]
    returns (loss, tb)

Why single core: on this part the collective subsystem costs ~60us per
execution (a ~44us start barrier plus AllGather trigger latency), while
the entire real workload is one 4MB logits read (~14us) plus small
matmuls.  Any multi-core split must pay the collective tax, so one core
with zero collectives wins by a wide margin.

Scoring: score(p) factors over a prefix/suffix split.  With
w[j,i] = prob[j, target[i]],
    score(p) = A[p[0:4]] + B[p[4:8]],
and argmax over all 40320 perms reduces to
    max_n (A[n] + maxB[complement(subset(n))]).
A and B are materialized directly in [70 subsets (partitions), 24
orderings (free)] layout via a rank factorization:
    A[s, o] = sum_j w[j, elems(s)[rankperm_o(j)]]
            = sum_{(rho,j)} W4[(rho,j), s] * OH24A[(rho,j), o]
where W4[(rho,j), s] = w[j, elems(s)[rho]] is itself one one-hot matmul
from the 64 gathered w values.  Everything downstream (per-subset max,
argmax, complement lookup) runs partition-parallel.

Because subsets are enumerated in lex order, complementation reverses
the order: comp(s) = 69 - s.  The complement lookup maxB[comp] is one
reversal matmul and cstar = 69 - sstar is pure arithmetic.

Index convention (HW-verified in the baseline kernel): r = i*8 + j
encodes the (i,j) pair of w[j, i] at SBUF partition r; j = r % 8,
i = (r % 64) // 8, half = r // 64.
"""

import numpy as np
from itertools import permutations, combinations

import concourse.bacc as bacc
import concourse.mybir as mybir
import concourse.tile as tile
from concourse.bass import IndirectOffsetOnAxis
from concourse.bass_utils import run_bass_kernel_spmd

M = 8
V = 128000
NT = 1680            # ordered distinct 4-tuples of 8 values
NS = 70              # 4-subsets of 8
NCHUNK = 8           # logits DMA/exp chunks
CH = V * M // 128 // NCHUNK   # 1000 cols per chunk on the [128, 8000] view

f32 = mybir.dt.float32
i32 = mybir.dt.int32
u8 = mybir.dt.uint8

AF = mybir.ActivationFunctionType
OP = mybir.AluOpType
AX = mybir.AxisListType

BIG = 1.0e9

# blob0: tables needed in the first microseconds (small => lands fast)
C_EYE = 0            # eye(70)                      [70, 70]
C_JH = 70            # jhmask (j,half selector)     [128, 16]
NBLOB0 = 86
# blob1: everything needed after the softmax reduction
C_JSEL = 0           # jsel (S_j row groups)        [128, 8]
C_OHW1 = 8           # elem(s, rank=half)           [128, 70]
C_OHW2 = 78          # elem(s, rank=2+half)         [128, 70]
C_O24A = 148         # rank one-hot, positions 0-3  [64, 24]
C_O24B = 172         # rank one-hot, positions 4-7  [64, 24]
C_OHC = 196          # reversal (complement) matrix [70, 70]
C_OIDX = 266         # oidx24[s,o] = o              [70, 24]
C_I70 = 290          # iota70 row                   [1, 70]
C_IV64 = 360         # i-index row (r//8)           [1, 64]
C_B4Q = 424          # {0,.25,.5,.75}x2 row         [1, 8]
C_REP8 = 432         # rep8[k, q] = [k == q%8]      [8, 64]
NBLOB1 = 496


def build_program(dbg=False):
    nc = bacc.Bacc("TRN2", target_bir_lowering=False, debug=False,
                   num_devices=1)

    # ---- I/O ----
    lgf = nc.dram_tensor("lgf", [M, V], f32, kind="ExternalInput").ap()
    lgf_ind = lgf.rearrange("j v -> (j v)").unsqueeze(1)
    blob0 = nc.dram_tensor("blob0", [128, NBLOB0], f32,
                           kind="ExternalInput").ap()
    blob1 = nc.dram_tensor("blob1", [128, NBLOB1], f32,
                           kind="ExternalInput").ap()
    # host-precomputed gather offsets: ioff[r] = (r%8)*V + target[r//8]
    ioff = nc.dram_tensor("ioff", [64, 1], i32, kind="ExternalInput").ap()
    tgtrow = nc.dram_tensor("tgtrow", [1, M], f32, kind="ExternalInput").ap()
    pfv = nc.dram_tensor("pfv", [NT, 4], u8, kind="ExternalInput").ap()
    o_out = nc.dram_tensor("out16", [1, 16], f32, kind="ExternalOutput").ap()

    with tile.TileContext(nc) as tc:
        with tc.tile_pool(name="sb", bufs=1) as sb, \
             tc.tile_pool(name="ps", bufs=1, space="PSUM") as ps:

            # ---------- stage in ----------
            # sync queue: logits chunks 0-3; gpsimd: the small tables
            # first (so they are not starved behind 4MB of logits), then
            # logits chunks 4-7.
            L = sb.tile([128, NCHUNK * CH], f32)          # [128, 8000]
            lgr_v = lgf.rearrange("j (s c) -> (j s) c", s=16)
            for k in range(NCHUNK // 2):
                nc.sync.dma_start(L[:, k * CH:(k + 1) * CH],
                                  lgr_v[:, k * CH:(k + 1) * CH])
            ioff_t = sb.tile([64, 1], i32)
            nc.gpsimd.dma_start(ioff_t[:], ioff)
            B0 = sb.tile([128, NBLOB0], f32)
            nc.gpsimd.dma_start(B0[:], blob0)
            # T'[j, i] = logits[j, target[i]] at partition r = i*8 + j
            T_p = sb.tile([64, 1], f32)
            nc.gpsimd.indirect_dma_start(
                T_p[:], None, lgf_ind,
                IndirectOffsetOnAxis(ap=ioff_t[:], axis=0))
            B = sb.tile([128, NBLOB1], f32)
            nc.gpsimd.dma_start(B[:], blob1)
            tgf = sb.tile([1, M], f32)
            nc.gpsimd.dma_start(tgf[:], tgtrow)
            for k in range(NCHUNK // 2, NCHUNK):
                nc.gpsimd.dma_start(L[:, k * CH:(k + 1) * CH],
                                    lgr_v[:, k * CH:(k + 1) * CH])

            one1 = B0[0:1, C_EYE:C_EYE + 1]
            eye8 = B0[0:M, C_EYE:C_EYE + M]
            eye64 = B0[0:64, C_EYE:C_EYE + 64]
            eye70 = B0[0:NS, C_EYE:C_EYE + NS]
            jh_v = B0[:, C_JH:C_JH + 16]
            jsel_v = B[:, C_JSEL:C_JSEL + M]
            ohw1_v = B[:, C_OHW1:C_OHW1 + NS]
            ohw2_v = B[:, C_OHW2:C_OHW2 + NS]
            o24a_v = B[0:64, C_O24A:C_O24A + 24]
            o24b_v = B[0:64, C_O24B:C_O24B + 24]
            ohc_v = B[0:NS, C_OHC:C_OHC + NS]
            oidx_v = B[0:NS, C_OIDX:C_OIDX + 24]
            i70_v = B[0:1, C_I70:C_I70 + NS]
            iv64_v = B[0:1, C_IV64:C_IV64 + 64]
            b4q_v = B[0:1, C_B4Q:C_B4Q + M]
            rep8_v = B[0:8, C_REP8:C_REP8 + 64]

            # dummy exp on zeroed scratch: pull the ACT exp-table load
            # into the DMA window
            scr8 = sb.tile([1, M], f32)
            nc.vector.memset(scr8[:], 0.0)
            nc.scalar.activation(scr8[:], scr8[:], AF.Exp)

            # ---------- softmax denominators (chunked exp) ----------
            E = sb.tile([128, NCHUNK * CH], f32)
            acc = sb.tile([128, NCHUNK], f32)

            def exp_chunk(k):
                nc.scalar.activation(E[:, k * CH:(k + 1) * CH],
                                     L[:, k * CH:(k + 1) * CH], AF.Exp,
                                     accum_out=acc[:, k:k + 1])

            exp_chunk(0)
            exp_chunk(1)

            Trow_ps = ps.tile([1, 64], f32, tag="t2")
            nc.tensor.matmul(Trow_ps[:], T_p[:], eye64, start=True, stop=True)
            Trow = sb.tile([1, 64], f32)
            nc.vector.tensor_copy(Trow[:], Trow_ps[:])
            # expTrow on the scalar queue after chunks 0-1: T is ready long
            # before chunk 1's exp retires, so the queue never stalls
            expTrow = sb.tile([1, 64], f32)
            nc.scalar.activation(expTrow[:], Trow_ps[:], AF.Exp)

            for k in range(2, NCHUNK):
                exp_chunk(k)

            # ---------- unnormalized W4 table (hidden under the DMA) ----
            # W4raw[(rho,j), s] = exp(T'[j, elems(s)[rho]])
            e128row = sb.tile([1, 128], f32)
            nc.vector.tensor_copy(
                e128row[:].rearrange("p (h r) -> p h r", h=2),
                expTrow[:].unsqueeze(1).to_broadcast((1, 2, 64)))
            eT128_ps = ps.tile([128, 1], f32, tag="t1")
            nc.tensor.matmul(eT128_ps[:], e128row[:], one1,
                             start=True, stop=True)
            eT128 = sb.tile([128, 1], f32)
            nc.vector.tensor_copy(eT128[:], eT128_ps[:])
            WJe = sb.tile([128, 16], f32)
            nc.vector.tensor_scalar(WJe[:], jh_v, eT128[:], None, OP.mult)
            psW1 = ps.tile([16, NS], f32, tag="w1")
            nc.tensor.matmul(psW1[:], WJe[:], ohw1_v, start=True, stop=True)
            psW2 = ps.tile([16, NS], f32, tag="w2")
            nc.tensor.matmul(psW2[:], WJe[:], ohw2_v, start=True, stop=True)
            Wraw = sb.tile([64, NS], f32)
            # rows 16-31 / 48-63 are dead but must be finite: OH24's zero
            # rows would still propagate NaN through the PE accumulate
            nc.vector.memset(Wraw[:], 0.0)
            nc.vector.tensor_copy(Wraw[0:16, :], psW1[:])
            nc.vector.tensor_copy(Wraw[32:48, :], psW2[:])

            # ---------- S_j, 1/S_j, log S_j ----------
            sums = sb.tile([128, 1], f32)
            nc.vector.tensor_reduce(sums[:], acc[:], axis=AX.X, op=OP.add)
            S8_ps = ps.tile([M, 1], f32, tag="t3")
            nc.tensor.matmul(S8_ps[:], jsel_v, sums[:], start=True, stop=True)
            S8sb = sb.tile([M, 1], f32)
            nc.vector.tensor_copy(S8sb[:], S8_ps[:])
            recipS_p = sb.tile([M, 1], f32)
            nc.vector.reciprocal(recipS_p[:], S8sb[:])
            S8row_ps = ps.tile([1, M], f32, tag="t4")
            nc.tensor.matmul(S8row_ps[:], S8sb[:], eye8, start=True, stop=True)
            lseN = sb.tile([1, M], f32)
            nc.scalar.activation(lseN[:], S8row_ps[:], AF.Ln)

            # ---------- normalize W4 and emit A/B in [70, 24] ----------
            rec64_ps = ps.tile([64, 1], f32, tag="t3")
            nc.tensor.matmul(rec64_ps[:], rep8_v, recipS_p[:],
                             start=True, stop=True)
            rec64 = sb.tile([64, 1], f32)
            nc.vector.tensor_copy(rec64[:], rec64_ps[:])
            W4 = sb.tile([64, NS], f32)
            nc.vector.tensor_scalar(W4[:], Wraw[:], rec64[:], None, OP.mult)
            psA = ps.tile([NS, 24], f32, tag="a70")
            nc.tensor.matmul(psA[:], W4[:], o24a_v, start=True, stop=True)
            psB = ps.tile([NS, 24], f32, tag="b70")
            nc.tensor.matmul(psB[:], W4[:], o24b_v, start=True, stop=True)

            # ---------- per-subset suffix max + first-argmax ----------
            maxB = sb.tile([NS, 1], f32)
            nc.vector.tensor_reduce(maxB[:], psB[:], axis=AX.X, op=OP.max)
            boE = sb.tile([NS, 24], f32)
            nc.vector.tensor_scalar(boE[:], psB[:], maxB[:], BIG,
                                    OP.is_lt, OP.mult)
            boE2 = sb.tile([NS, 24], f32)
            bo = sb.tile([NS, 1], f32)
            nc.vector.tensor_tensor(boE2[:], boE[:], oidx_v, OP.add)
            nc.vector.tensor_reduce(bo[:], boE2[:], axis=AX.X, op=OP.min)
            # maxBc[s] = maxB[69 - s]
            mBc_ps = ps.tile([NS, 1], f32, tag="t3")
            nc.tensor.matmul(mBc_ps[:], ohc_v, maxB[:], start=True, stop=True)
            maxBc = sb.tile([NS, 1], f32)
            nc.vector.tensor_copy(maxBc[:], mBc_ps[:])

            # ---------- tot = A + maxBc; row maxima and argmaxes ----------
            tot = sb.tile([NS, 24], f32)
            nc.vector.tensor_scalar(tot[:], psA[:], maxBc[:], None, OP.add)
            rmax = sb.tile([NS, 1], f32)
            nc.vector.tensor_reduce(rmax[:], tot[:], axis=AX.X, op=OP.max)
            oE = sb.tile([NS, 24], f32)
            nc.vector.tensor_scalar(oE[:], tot[:], rmax[:], BIG,
                                    OP.is_lt, OP.mult)
            oE2 = sb.tile([NS, 24], f32)
            oarg = sb.tile([NS, 1], f32)
            nc.vector.tensor_tensor(oE2[:], oE[:], oidx_v, OP.add)
            nc.vector.tensor_reduce(oarg[:], oE2[:], axis=AX.X, op=OP.min)

            # cross-partition: transpose the three [70,1] columns to rows
            rmT_ps = ps.tile([1, NS], f32, tag="w1")
            nc.tensor.matmul(rmT_ps[:], rmax[:], eye70, start=True, stop=True)
            oaT_ps = ps.tile([1, NS], f32, tag="t1")
            nc.tensor.matmul(oaT_ps[:], oarg[:], eye70, start=True, stop=True)
            boT_ps = ps.tile([1, NS], f32, tag="t2")
            nc.tensor.matmul(boT_ps[:], bo[:], eye70, start=True, stop=True)
            rmT = rmT_ps[:]
            oaT = oaT_ps[:]
            boT = boT_ps[:]

            # global first-max over subsets
            mfin = sb.tile([1, 1], f32)
            nc.vector.tensor_reduce(mfin[:], rmT, axis=AX.X, op=OP.max)
            es = sb.tile([1, NS], f32)
            nc.vector.tensor_scalar(es[:], rmT, mfin[:], BIG,
                                    OP.is_lt, OP.mult)
            es2 = sb.tile([1, NS], f32)
            sstar = sb.tile([1, 1], f32)
            nc.vector.tensor_tensor(es2[:], es[:], i70_v, OP.add)
            nc.vector.tensor_reduce(sstar[:], es2[:], axis=AX.X, op=OP.min)

            # ostar = oarg[sstar]; cstar = 69 - sstar; bostar = bo[cstar]
            eq1 = sb.tile([1, NS], f32)
            nc.vector.tensor_scalar(eq1[:], i70_v, sstar[:], None, OP.is_equal)
            g1 = sb.tile([1, NS], f32)
            ostar = sb.tile([1, 1], f32)
            nc.vector.tensor_tensor(g1[:], eq1[:], oaT, OP.mult)
            nc.vector.tensor_reduce(ostar[:], g1[:], axis=AX.X, op=OP.add)
            cstar = sb.tile([1, 1], f32)
            nc.vector.tensor_scalar(cstar[:], sstar[:], -1.0, 69.0,
                                    OP.mult, OP.add)  # imm scalars only
            eq2 = sb.tile([1, NS], f32)
            nc.vector.tensor_scalar(eq2[:], i70_v, cstar[:], None, OP.is_equal)
            g3 = sb.tile([1, NS], f32)
            bostar = sb.tile([1, 1], f32)
            nc.vector.tensor_tensor(g3[:], eq2[:], boT, OP.mult)
            nc.vector.tensor_reduce(bostar[:], g3[:], axis=AX.X, op=OP.add)

            # ---------- winning tuple indices -> byte offsets ----------
            # naRAW = sstar*24 + ostar ; offf[0:4] = (b4q + naRAW)*4
            naRAW = sb.tile([1, 1], f32)
            nc.vector.tensor_scalar(naRAW[:], sstar[:], 24.0, ostar[:],
                                    OP.mult, OP.add)
            nbRAW = sb.tile([1, 1], f32)
            nc.vector.tensor_scalar(nbRAW[:], cstar[:], 24.0, bostar[:],
                                    OP.mult, OP.add)
            offf = sb.tile([1, M], f32)
            nc.vector.tensor_scalar(offf[:, 0:4], b4q_v[:, 0:4], naRAW[:],
                                    4.0, OP.add, OP.mult)
            nc.vector.tensor_scalar(offf[:, 4:8], b4q_v[:, 4:8], nbRAW[:],
                                    4.0, OP.add, OP.mult)
            offp_ps = ps.tile([M, 1], f32, tag="t3")
            nc.tensor.matmul(offp_ps[:], offf[:], one1, start=True, stop=True)
            offp = sb.tile([M, 1], i32)
            nc.vector.tensor_copy(offp[:], offp_ps[:])
            pb8 = sb.tile([M, 1], u8)
            nc.gpsimd.indirect_dma_start(
                pb8[:], None, pfv.rearrange("a b -> (a b)").unsqueeze(1),
                IndirectOffsetOnAxis(ap=offp[:], axis=0))
            pb8f = sb.tile([M, 1], f32)
            nc.vector.tensor_copy(pb8f[:], pb8[:])
            pbf_ps = ps.tile([1, M], f32, tag="t4")
            nc.tensor.matmul(pbf_ps[:], pb8f[:], eye8, start=True, stop=True)
            pbf = sb.tile([1, M], f32)
            nc.vector.tensor_copy(pbf[:], pbf_ps[:])

            # ---------- loss and tb for the winning assignment ----------
            # mask[r] = (i(r) == perm_best[j(r)]),  r = i*8 + j
            mask = sb.tile([1, 64], f32)
            nc.vector.tensor_tensor(
                mask[:].rearrange("p (i j) -> p i j", j=8),
                iv64_v.rearrange("p (i j) -> p i j", j=8),
                pbf[:].unsqueeze(1).to_broadcast((1, 8, 8)), OP.is_equal)
            tm = sb.tile([1, 64], f32)
            nc.vector.tensor_tensor(tm[:], mask[:], Trow[:], OP.mult)
            Tb = sb.tile([1, M], f32)
            nc.vector.tensor_reduce(Tb[:],
                                    tm[:].rearrange("p (i j) -> p j i", j=8),
                                    axis=AX.X, op=OP.add)
            out16 = sb.tile([1, 16], f32)
            nc.vector.tensor_tensor(out16[:, 0:M], lseN[:], Tb[:],
                                    OP.subtract)

            tm2 = sb.tile([1, 64], f32)
            nc.vector.tensor_tensor(
                tm2[:].rearrange("p (i j) -> p i j", j=8),
                mask[:].rearrange("p (i j) -> p i j", j=8),
                tgf[:].unsqueeze(2).to_broadcast((1, 8, 8)), OP.mult)
            nc.vector.tensor_reduce(out16[:, M:16],
                                    tm2[:].rearrange("p (i j) -> p j i", j=8),
                                    axis=AX.X, op=OP.add)

            nc.sync.dma_start(o_out, out16[:])

            if dbg:
                def dump(name, t, shape):
                    o = nc.dram_tensor(name, shape, t.dtype,
                                       kind="ExternalOutput").ap()
                    nc.sync.dma_start(o, t)
                dump("d_sums", sums[:], [128, 1])
                dump("d_Trow", Trow[:], [1, 64])
                dump("d_W4", W4[:], [64, NS])
                dump("d_maxB", maxB[:], [NS, 1])
                dump("d_bo", bo[:], [NS, 1])
                dump("d_maxBc", maxBc[:], [NS, 1])
                dump("d_sstar", sstar[:], [1, 1])
                dump("d_ostar", ostar[:], [1, 1])
                dump("d_cstar", cstar[:], [1, 1])
                dump("d_bostar", bostar[:], [1, 1])
                dump("d_offf", offf[:], [1, M])
                dump("d_pbf", pbf[:], [1, M])

    nc.compile()
    return nc


_NC_CACHE = None


def _get_program():
    global _NC_CACHE
    if _NC_CACHE is None:
        _NC_CACHE = build_program()
    return _NC_CACHE


def _make_tables():
    subsets = list(combinations(range(8), 4))            # 70, lex order
    elems = np.array(subsets, dtype=np.int64)            # [70, 4] sorted
    rp = np.array(list(permutations(range(4))), dtype=np.int64)  # [24, 4]
    tuples = []
    for s in subsets:
        for t in permutations(s):
            tuples.append(t)
    tuples = np.array(tuples, dtype=np.int64)            # [1680, 4]

    r = np.arange(128)
    jr = r % 8
    ir = (r % 64) // 8
    hr = r // 64

    blob0 = np.zeros((128, NBLOB0), dtype=np.float32)
    blob0[0:NS, C_EYE:C_EYE + NS] = np.eye(NS, dtype=np.float32)
    # jhmask[r, q] = [j(r) == q%8] * [half(r) == q//8]
    q = np.arange(16)
    blob0[:, C_JH:C_JH + 16] = (
        (jr[:, None] == (q % 8)[None, :]) & (hr[:, None] == (q // 8)[None, :]))

    blob = np.zeros((128, NBLOB1), dtype=np.float32)
    blob[:, C_JSEL:C_JSEL + M] = (
        np.arange(128)[:, None] // 16 == np.arange(8)[None, :])
    # OHW1[r, s] = [i(r) == elems(s)[half(r)]]
    blob[:, C_OHW1:C_OHW1 + NS] = (ir[:, None] == elems[:, 0:2].T[hr, :])
    # OHW2[r, s] = [i(r) == elems(s)[2 + half(r)]]
    blob[:, C_OHW2:C_OHW2 + NS] = (ir[:, None] == elems[:, 2:4].T[hr, :])
    # OH24A/B rows: W4 row layout r<16: rho=r//8, j=r%8;
    #               32<=r<48: rho=2+(r-32)//8, j=(r-32)%8; else dead.
    o24a = np.zeros((64, 24), dtype=np.float32)
    o24b = np.zeros((64, 24), dtype=np.float32)
    for rr in range(64):
        if rr < 16:
            rho, j = rr // 8, rr % 8
        elif 32 <= rr < 48:
            rho, j = 2 + (rr - 32) // 8, (rr - 32) % 8
        else:
            continue
        if j <= 3:
            o24a[rr, :] = (rp[:, j] == rho)
        else:
            o24b[rr, :] = (rp[:, j - 4] == rho)
    blob[0:64, C_O24A:C_O24A + 24] = o24a
    blob[0:64, C_O24B:C_O24B + 24] = o24b
    # reversal matrix: ohc[k, s] = [k == 69 - s]
    blob[0:NS, C_OHC:C_OHC + NS] = np.eye(NS, dtype=np.float32)[::-1]
    blob[0:NS, C_OIDX:C_OIDX + 24] = np.arange(24)[None, :]
    blob[0, C_I70:C_I70 + NS] = np.arange(NS)
    blob[0, C_IV64:C_IV64 + 64] = np.arange(64) // 8
    blob[0, C_B4Q:C_B4Q + M] = [0.0, 0.25, 0.5, 0.75] * 2
    blob[0:8, C_REP8:C_REP8 + 64] = (
        np.arange(8)[:, None] == (np.arange(64) % 8)[None, :])

    return blob0, blob, tuples.astype(np.uint8)


_TABLES = None


def make_in_maps(logits, target, perms):
    global _TABLES
    if _TABLES is None:
        _TABLES = _make_tables()
    blob0, blob1, pfv = _TABLES
    logits = np.ascontiguousarray(np.asarray(logits, dtype=np.float32))
    target = np.asarray(target).astype(np.int64).reshape(M)
    r = np.arange(64)
    ioff = ((r % 8) * V + target[r // 8]).astype(np.int32).reshape(64, 1)
    tgtrow = target.astype(np.float32).reshape(1, M)
    return [{"lgf": logits, "blob0": blob0, "blob1": blob1, "ioff": ioff,
             "tgtrow": tgtrow, "pfv": pfv}]


def run(logits, target, perms, trace=False):
    nc = _get_program()
    in_maps = make_in_maps(logits, target, perms)
    res = run_bass_kernel_spmd(nc, in_maps, core_ids=[0], trace=trace)
    out16 = res.results[0]["out16"].reshape(16)
    loss = out16[0:M].astype(np.float32)
    tb = np.rint(out16[M:16]).astype(np.int32)
    return loss, tb, res


def kernel(logits, target, perms):
    loss, tb, _ = run(logits, target, perms, trace=False)
    return loss, tb
